# revision 1
# baseline (speedup 1.0000x reference)
"""Trainium2 Bass kernel for nn_AttentionHead (B=4, n_ctx=4096, d_model=1024,
d_hidden=64, causal, scale=1/sqrt(d_model)).

Sharding: 8 cores = 4 batches x 2 balanced causal shards. Core (b, s) handles
the 2048 query rows in 64-row chunks with chunk%2 == s. Keys/x-columns are
permuted per core (my-parity chunks first within each 512-key ntile) so that
every core runs the IDENTICAL SPMD program:

  - slot j (0..3) = 512 queries = my chunks of ntiles 2j, 2j+1
  - slot j attends k-tiles t = 0..8(j+1)-1 (128 permuted keys each)
  - k-tiles t < 8j are fully open; t = 8j + r (r in 0..7) get an additive
    causal mask that depends only on (r, s) -> 8 mask tiles per core, sent
    as data.

Per-core pipeline (all matmuls in float32r: 1 cycle/row at N>=256):
  A: KT/VT = [Wk;Wov] @ xT (weights stationary, PSUM-accumulated over 8
     d_model chunks, biases folded in as K=1 rank-1 matmuls against a ones
     row); Q likewise from each ntile's first 256 columns (= my 4 chunks).
     V transposed to natural [k,64] layout via PE transpose, with an
     appended ones column so attn@[V,1] also yields the softmax denominator.
  B: S^T[k,q] = KT_tile^T @ QT_slot -> PSUM (two k-tiles paired per 2-bank
     PSUM tile); additive mask via identity matmul for diagonal tiles;
     exp((S+M)/32) on ACT over the pair -> SBUF (no row-max subtraction
     needed: |scores/32| <~ 1.5).
  C: O65 += V65_tile^T @ E (PSUM accumulate over k-tiles); row 64 = denom.
  D: y_tile = (O65_slice^T @ [Wo^T; bo]) * (1/den) -- den row makes the
     matmul add den*bo, so the per-partition reciprocal multiply yields
     O@Wo^T/den + bo exactly. Reciprocals come from 16 PE transposes of the
     denominator row into [128,16] + one DVE reciprocal.

DMA instruction count is kept low (each DMA holds the shared HWDGE
descriptor generator ~625ns): x arrives bf16 in an ntile-major host layout
(two [128,4x512] loads per 512-key ntile, so each ntile's projections and
the attention slots that depend on them cascade right behind the DMA
stream), y leaves as 16 [128,1024] stores, constants are consolidated
single loads ordered by first use. Emission is interleaved (A ntiles,
then each slot's B/C with the previous slot's D inside) and C trails one
pair behind B/exp so no engine's in-order stream blocks on another.
"""

import math

import numpy as np

D = 1024
H = 64
N = 4096
B = 4
CH = 64  # query chunk size (rows)
NT = 8  # ntiles of 512 keys
NEG = -1e10
# per masked-tile r: length of the fully-dead leading q-column prefix, min
# over both core parities, clamped to 256 (fp32r full-rate floor)
MASK_OFFS = [0, 128, 0, 128, 256, 256, 256, 256]
# per masked-tile r: end of the nonzero mask band (max over parities); the
# mask-add matmul only needs to cover [MASK_OFFS[r], MASK_ENDS[r])
MASK_ENDS = [127, 255, 128, 256, 383, 511, 384, 512]

_PROG = None  # cached compiled program


# ---------------------------------------------------------------- host layout


def _key_order(s: int) -> np.ndarray:
    order = []
    for n in range(NT):
        mine = [8 * n + t for t in range(8) if t % 2 == s]
        theirs = [8 * n + t for t in range(8) if t % 2 != s]
        for c in mine + theirs:
            order.extend(range(CH * c, CH * c + CH))
    return np.array(order)


def _masks(s: int) -> np.ndarray:
    ko = _key_order(s)
    qo = np.array([CH * c + i for c in range(s, 64, 2) for i in range(CH)])
    m = np.zeros((8, 128, 512), dtype=np.float32)
    for r in range(8):
        keys = ko[128 * r : 128 * (r + 1)]
        qs = qo[0:512]
        m[r] = np.where(keys[:, None] <= qs[None, :], 0.0, NEG)
    return m


# ---------------------------------------------------------------- bass program


def _build():
    import concourse.mybir as mybir
    import concourse.tile as tile
    from concourse import bacc

    f32 = mybir.dt.float32
    f32r = mybir.dt.float32r
    bf16 = mybir.dt.bfloat16

    nc = bacc.Bacc("TRN2", target_bir_lowering=False, debug=False, num_devices=8)

    xh = nc.dram_tensor("xh", [NT, 128, 8, 512], bf16, kind="ExternalInput").ap()
    wkv = nc.dram_tensor("wkv", [9, 128, 128], bf16, kind="ExternalInput").ap()
    wq = nc.dram_tensor("wq", [9, 128, 64], bf16, kind="ExternalInput").ap()
    wobo = nc.dram_tensor("wobo", [65, 1024], f32r, kind="ExternalInput").ap()
    masks = nc.dram_tensor("masks", [8, 128, 512], bf16, kind="ExternalInput").ap()
    identb = nc.dram_tensor("identb", [128, 128], bf16, kind="ExternalInput").ap()
    ident = nc.dram_tensor("ident", [128, 128], f32r, kind="ExternalInput").ap()
    biases = nc.dram_tensor("biases", [128, 2], f32, kind="ExternalInput").ap()
    vones = nc.dram_tensor("vones", [128, 32, 1], f32r, kind="ExternalInput").ap()
    y = nc.dram_tensor("y", [2048, 1024], f32, kind="ExternalOutput").ap()

    Exp = mybir.ActivationFunctionType.Exp
    Identity = mybir.ActivationFunctionType.Identity
    mult = mybir.AluOpType.mult
    add_op = mybir.AluOpType.add
    scale = 1.0 / math.sqrt(D)

    with tile.TileContext(nc) as tc:
        with (
            tc.tile_pool(name="consts", bufs=1) as consts,
            tc.tile_pool(name="xp", bufs=4) as xpool,
            tc.tile_pool(name="ep", bufs=8) as epool,
            tc.tile_pool(name="yp", bufs=4) as ypool,
            tc.tile_pool(name="pkv", bufs=1, space="PSUM") as pkv,
            tc.tile_pool(name="pq", bufs=1, space="PSUM") as pq,
            tc.tile_pool(name="po", bufs=2, space="PSUM") as po,
            tc.tile_pool(name="ps", bufs=2, space="PSUM") as ps,
        ):
            # ---- constants (one DMA each)
            wkv_sb = consts.tile([128, 9 * 128], bf16)
            nc.gpsimd.dma_start(
                wkv_sb[:].rearrange("p (c f) -> p c f", c=9),
                wkv.rearrange("c p f -> p c f"),
            )
            wq_sb = consts.tile([128, 9 * 64], bf16)
            nc.gpsimd.dma_start(
                wq_sb[:].rearrange("p (c f) -> p c f", c=9),
                wq.rearrange("c p f -> p c f"),
            )
            id_sb = consts.tile([128, 128], f32r)
            nc.gpsimd.dma_start(id_sb[:], ident[:])
            idb_sb = consts.tile([128, 128], bf16)
            nc.gpsimd.dma_start(idb_sb[:], identb[:])
            bias_sb = consts.tile([128, 2], f32)  # col 0: [bk|bov], col 1: bq
            nc.gpsimd.dma_start(bias_sb[:], biases[:])

            kvt_sb = consts.tile([128, N], f32r)  # rows 0:64 KT, 64:128 VT
            qt_sb = consts.tile([H, 2048], f32r)
            v65_sb = consts.tile([128, 32 * 65], f32r)
            nc.gpsimd.dma_start(
                v65_sb[:].rearrange("p (t c) -> p t c", c=65)[:, :, 64:65], vones[:]
            )
            mask_sb = consts.tile([128, 8 * 512], bf16)
            nc.gpsimd.dma_start(
                mask_sb[:].rearrange("p (m f) -> p m f", m=8),
                masks.rearrange("m p f -> p m f"),
            )
            wobo_sb = consts.tile([65, 1024], f32r)
            nc.gpsimd.dma_start(wobo_sb[:], wobo[:])
            ot_sb = consts.tile([65, 2048], f32r)
            recip_sb = consts.tile([128, 16], f32)
            scratch_sb = consts.tile([1, 8], f32)

            # prewarm the ACT exp table while DMAs stream
            nc.scalar.activation(
                scratch_sb[:], id_sb[0:1, 0:8].bitcast(f32), Exp, bias=0.0, scale=1.0
            )

            # ---- interleaved emission: stage A ntiles, with slot j's
            # B/C/D emitted right after ntile 2j+1 so each engine's in-order
            # instruction stream matches data-readiness order.
            def emit_a(n):
                xn = xpool.tile([128, 4096], bf16, tag="x")
                xnv = xn[:].rearrange("p (c f) -> p c f", c=8)
                nc.sync.dma_start(xnv[:], xh[n])
                kvp = pkv.tile([128, 512], f32, tag="kv")
                qp = pq.tile([64, 256], f32, tag="q")
                for c in range(8):
                    nc.tensor.matmul(
                        kvp[:],
                        wkv_sb[:, 128 * c : 128 * (c + 1)],
                        xn[:, 512 * c : 512 * c + 512],
                        start=(c == 0),
                        stop=(c == 7),
                    )
                    nc.tensor.matmul(
                        qp[:],
                        wq_sb[:, 64 * c : 64 * (c + 1)],
                        xn[:, 512 * c : 512 * c + 256],
                        start=(c == 0),
                        stop=(c == 7),
                    )
                nc.vector.tensor_scalar(
                    out=kvt_sb[:, 512 * n : 512 * (n + 1)],
                    in0=kvp[:],
                    scalar1=bias_sb[:, 0:1],
                    scalar2=None,
                    op0=add_op,
                )
                nc.vector.tensor_scalar(
                    out=qt_sb[:, 256 * n : 256 * (n + 1)],
                    in0=qp[:],
                    scalar1=bias_sb[0:64, 1:2],
                    scalar2=None,
                    op0=add_op,
                )
                for t in range(4 * n, 4 * n + 4):
                    vp = po.tile([128, 64], f32r, tag="o")
                    nc.tensor.transpose(
                        vp[:],
                        kvt_sb[64:128, 128 * t : 128 * (t + 1)],
                        id_sb[64:128, 64:128],
                    )
                    nc.vector.tensor_copy(v65_sb[:, 65 * t : 65 * t + 64], vp[:])

            def emit_d(i, wide=False):
                ys = ypool.tile([128, 1024], f32, tag="y")
                if wide:
                    # tail D's: the S rotation is free, use its 2-bank slots
                    # so consecutive i's pipeline instead of serializing on kv
                    yp = ps.tile([128, 1024], f32, tag="s")
                    for d in range(2):
                        nc.tensor.matmul(
                            yp[:, 512 * d : 512 * (d + 1)],
                            ot_sb[:, 128 * i : 128 * (i + 1)],
                            wobo_sb[:, 512 * d : 512 * (d + 1)],
                            start=True,
                            stop=True,
                        )
                    nc.vector.tensor_scalar(
                        out=ys[:],
                        in0=yp[:],
                        scalar1=recip_sb[:, i : i + 1],
                        scalar2=None,
                        op0=mult,
                    )
                else:
                    for d in range(2):
                        yp = pkv.tile([128, 512], f32, tag="kv")
                        nc.tensor.matmul(
                            yp[:],
                            ot_sb[:, 128 * i : 128 * (i + 1)],
                            wobo_sb[:, 512 * d : 512 * (d + 1)],
                            start=True,
                            stop=True,
                        )
                        nc.vector.tensor_scalar(
                            out=ys[:, 512 * d : 512 * (d + 1)],
                            in0=yp[:],
                            scalar1=recip_sb[:, i : i + 1],
                            scalar2=None,
                            op0=mult,
                        )
                nc.gpsimd.dma_start(y[128 * i : 128 * (i + 1), :], ys[:])

            pending_d = []

            bc_state = {}

            def emit_bc(j, t_lo=0, t_hi=None, finish=True):
                nk = 8 * (j + 1)
                if t_hi is None:
                    t_hi = nk
                if j in bc_state:
                    op_ = bc_state[j]
                else:
                    op_ = po.tile([65, 512], f32, tag="o")
                    bc_state[j] = op_
                def emit_c(t0, et, offs):
                    for h in range(2):
                        t = t0 + h
                        off = offs[h]
                        nc.tensor.matmul(
                            op_[:, off:512],
                            v65_sb[:, 65 * t : 65 * (t + 1)],
                            et[:, 512 * h + off : 512 * (h + 1)],
                            start=(t == 0),
                            stop=(t == nk - 1),
                        )

                # software-pipelined: C trails one pair behind B/exp so the
                # PE stream never waits on the exp of the pair it just fed
                prevs = []
                for t0 in range(t_lo, t_hi, 2):
                    if pending_d and t0 % 4 == 0 and t0 > 0:
                        emit_d(pending_d.pop(0))
                    sp = ps.tile([128, 1024], f32, tag="s")
                    offs = []
                    for h in range(2):
                        t = t0 + h
                        rr = t - 8 * j
                        # leading q-columns of a masked tile that are fully
                        # causally dead for BOTH core parities (clamped to 256
                        # so fp32r keeps its N>=256 full rate) - see MASK_OFFS
                        off = 0 if rr < 0 else MASK_OFFS[rr]
                        offs.append(off)
                        nc.tensor.matmul(
                            sp[:, 512 * h + off : 512 * (h + 1)],
                            kvt_sb[0:64, 128 * t : 128 * (t + 1)],
                            qt_sb[:, 512 * j + off : 512 * (j + 1)],
                            start=True,
                            stop=(rr < 0),
                        )
                        if rr >= 0:
                            end = MASK_ENDS[rr]
                            nc.tensor.matmul(
                                sp[:, 512 * h + off : 512 * h + end],
                                idb_sb[:],
                                mask_sb[:, 512 * rr + off : 512 * rr + end],
                                start=False,
                                stop=True,
                            )
                    et = epool.tile([128, 1024], f32r, tag="e")
                    if offs == [0, 0]:
                        nc.scalar.activation(et[:], sp[:], Exp, bias=0.0, scale=scale)
                    elif offs[0] == offs[1]:
                        # one strided activation over both halves' live regions
                        o = offs[0]
                        nc.scalar.activation(
                            et[:].rearrange("p (b f) -> p b f", b=2)[:, :, o:512],
                            sp[:].rearrange("p (b f) -> p b f", b=2)[:, :, o:512],
                            Exp,
                            bias=0.0,
                            scale=scale,
                        )
                    else:
                        for h in range(2):
                            o = 512 * h + offs[h]
                            nc.scalar.activation(
                                et[:, o : 512 * (h + 1)],
                                sp[:, o : 512 * (h + 1)],
                                Exp,
                                bias=0.0,
                                scale=scale,
                            )
                    prevs.append((t0, et, offs))
                    if len(prevs) > 2:
                        emit_c(*prevs.pop(0))
                for p in prevs:
                    emit_c(*p)
                if not finish:
                    return
                nc.vector.tensor_copy(ot_sb[:, 512 * j : 512 * (j + 1)], op_[:])
                rp = pq.tile([128, 4], f32, tag="q")
                for ii in range(4):
                    i = 4 * j + ii
                    nc.tensor.transpose(
                        rp[:, ii : ii + 1],
                        ot_sb[64:65, 128 * i : 128 * (i + 1)].bitcast(f32),
                        id_sb[64:65, 64:65].bitcast(f32),
                    )
                nc.vector.reciprocal(recip_sb[:, 4 * j : 4 * j + 4], rp[:])
                pending_d.extend(range(4 * j, 4 * j + 4))

            for n in range(NT):
                emit_a(n)
                if n % 2 == 1:
                    emit_bc((n - 1) // 2)

            for i in pending_d:
                emit_d(i, wide=True)

    nc.compile()
    return nc


def _get_prog():
    global _PROG
    if _PROG is None:
        _PROG = _build()
    return _PROG


# ---------------------------------------------------------------- entry point


def _xh(xb, korder):
    """[ntile, partition, chunk, 512] bf16 layout of x[b][korder].T."""
    import ml_dtypes

    xt = xb[korder].T  # [1024, 4096]
    return np.ascontiguousarray(
        xt.reshape(8, 128, 8, 512).transpose(2, 1, 0, 3).astype(ml_dtypes.bfloat16)
    )


def kernel(x, Wq, bq, Wk, bk, Wov, bov, Wo, bo, _trace=False):
    from concourse import bass_utils

    x = np.ascontiguousarray(np.asarray(x, dtype=np.float32))
    Wq = np.asarray(Wq, dtype=np.float32)
    bq = np.asarray(bq, dtype=np.float32)
    Wk = np.asarray(Wk, dtype=np.float32)
    bk = np.asarray(bk, dtype=np.float32)
    Wov = np.asarray(Wov, dtype=np.float32)
    bov = np.asarray(bov, dtype=np.float32)
    Wo = np.asarray(Wo, dtype=np.float32)
    bo = np.asarray(bo, dtype=np.float32)

    nc = _get_prog()

    wkv_arr = np.zeros((9, 128, 128), dtype=np.float32)
    wkv_t = np.concatenate([Wk, Wov], axis=0).T  # [1024, 128]
    for c in range(8):
        wkv_arr[c] = wkv_t[128 * c : 128 * (c + 1)]
    wkv_arr[8][0] = np.concatenate([bk, bov])

    wq_arr = np.zeros((9, 128, 64), dtype=np.float32)
    wq_t = Wq.T  # [1024, 64]
    for c in range(8):
        wq_arr[c] = wq_t[128 * c : 128 * (c + 1)]
    wq_arr[8][0] = bq

    import ml_dtypes

    wobo_arr = np.concatenate([Wo.T, bo[None, :]], axis=0)  # [65, 1024]
    wkv_arr = wkv_arr.astype(ml_dtypes.bfloat16)
    wq_arr = wq_arr.astype(ml_dtypes.bfloat16)
    biases_arr = np.zeros((128, 2), dtype=np.float32)
    biases_arr[:, 0] = np.concatenate([bk, bov])
    biases_arr[0:64, 1] = bq
    ident_arr = np.eye(128, dtype=np.float32)
    masks_s = [_masks(0), _masks(1)]
    korder_s = [_key_order(0), _key_order(1)]

    in_maps = []
    for core in range(8):
        b, s = divmod(core, 2)
        in_maps.append(
            {
                "xh": _xh(x[b], korder_s[s]),
                "wkv": wkv_arr,
                "wq": wq_arr,
                "wobo": wobo_arr,
                "masks": masks_s[s].astype(ml_dtypes.bfloat16),
                "identb": ident_arr.astype(ml_dtypes.bfloat16),
                "ident": ident_arr,
                "biases": biases_arr,
                "vones": np.ones((128, 32, 1), dtype=np.float32),
            }
        )

    res = bass_utils.run_bass_kernel_spmd(
        nc, in_maps, core_ids=list(range(8)), trace=_trace
    )

    y = np.empty((B, N, D), dtype=np.float32)
    for core in range(8):
        b, s = divmod(core, 2)
        y[b].reshape(64, CH, D)[s::2] = res.results[core]["y"].reshape(32, CH, D)
    return y



# revision 3
# speedup vs baseline: 1.0183x; 1.0183x over previous
"""Trainium2 Bass kernel for nn_AttentionHead (B=4, n_ctx=4096, d_model=1024,
d_hidden=64, causal, scale=1/sqrt(d_model)).

Sharding: 8 cores = 4 batches x 2 balanced causal shards. Core (b, s) handles
the 2048 query rows in 64-row chunks with chunk%2 == s. Keys/x-columns are
permuted per core (my-parity chunks first within each 512-key ntile) so that
every core runs the IDENTICAL SPMD program:

  - slot j (0..3) = 512 queries = my chunks of ntiles 2j, 2j+1
  - slot j attends k-tiles t = 0..8(j+1)-1 (128 permuted keys each)
  - k-tiles t < 8j are fully open; t = 8j + r (r in 0..7) get an additive
    causal mask that depends only on (r, s) -> 8 mask tiles per core, sent
    as data.

v2 design notes (cost model: matmul cost = moving-dim rows only; DMA
transfers from different issuing engines overlap; same-engine serialize):

  A: KT/VT = [Wk;Wov] @ xT fused (PSUM-accum over 8 d_model chunks), Q
     likewise; PSUM->SBUF copies add biases and downcast to bf16 (bf16
     matmuls are full rate at any N, unlike f32r's N>=256). V transposed
     to natural [k,64] bf16 layout via PE transpose; v65 keeps an
     appended ones column (DVE memset) so E^T@[V|1] also yields the
     softmax denominator.
  B: S^T[k,q] = KT_tile^T @ QT_slot -> PSUM pair tile; additive mask via
     identity matmul for diagonal tiles; exp((S+M)/32) on ACT -> bf16 E.
  C (transposed vs v1): O[q,65] += E_chunk^T @ V65_tile, i.e. E is the
     stationary operand and the 65-wide V65 is moving: 65 rows/chunk-tile
     instead of 512/tile -- less than half the PE cost of v1's C. Col 64
     accumulates the denominator per q-partition.
  D: per 128-q chunk: recip = 1/O[:,64] (DVE), normalize-copy
     O*recip -> bf16 (fused into the mandatory PSUM evacuation; makes
     col 64 exactly 1.0), PE-transpose to OT[65,128], Pool-copy to SBUF,
     then y = OT^T @ [Wo^T; bo] -- the 1.0 row adds bo exactly, so no
     per-element recip multiply is needed after the matmul. y is copied
     to bf16 (DVE/Pool alternating) and stored per-slot.

DMA: three independent streams. SP carries x ntiles (x0 split in half so
PE starts ~1.5us earlier); ACT carries the consts blob + x1/x3 + masks +
wobo (interleaved so each lands just before first use); Pool carries the
4 per-slot bf16 y stores (SWDGE). y is written bf16 and upcast on host
(adds ~0.2% fro error vs the 2e-2 budget). A few warmup matmuls on the
consts blob ramp the PE p-state during the DMA fill.
"""

import math

import numpy as np

D = 1024
H = 64
N = 4096
B = 4
CH = 64  # query chunk size (rows)
NT = 8  # ntiles of 512 keys
NEG = -1e10

_PROG = None  # cached compiled program
_META = None  # cached mask offsets/ends


# ---------------------------------------------------------------- host layout


def _key_order(s: int) -> np.ndarray:
    order = []
    for n in range(NT):
        mine = [8 * n + t for t in range(8) if t % 2 == s]
        theirs = [8 * n + t for t in range(8) if t % 2 != s]
        for c in mine + theirs:
            order.extend(range(CH * c, CH * c + CH))
    return np.array(order)


def _masks(s: int) -> np.ndarray:
    ko = _key_order(s)
    qo = np.array([CH * c + i for c in range(s, 64, 2) for i in range(CH)])
    m = np.zeros((8, 128, 512), dtype=np.float32)
    for r in range(8):
        keys = ko[128 * r : 128 * (r + 1)]
        qs = qo[0:512]
        m[r] = np.where(keys[:, None] <= qs[None, :], 0.0, NEG)
    return m


def _mask_meta():
    """Per masked-tile r (min/max over both parities so the shared program is
    valid for either): OFF_C = 128-floored fully-dead q-prefix, END = end of
    the nonzero mask band."""
    global _META
    if _META is not None:
        return _META
    offs = []
    ends = []
    ms = [_masks(0), _masks(1)]
    for r in range(8):
        offr, endr = [], []
        for s in (0, 1):
            dead = ms[s][r] != 0.0
            colall = dead.all(axis=0)
            off = 0
            while off < 512 and colall[off]:
                off += 1
            anyd = dead.any(axis=1).any()
            cols = np.nonzero(dead.any(axis=0))[0]
            end = int(cols.max()) + 1 if cols.size else 0
            offr.append(off)
            endr.append(end)
        offs.append((min(offr) // 128) * 128)
        ends.append(max(endr))
    _META = (offs, ends)
    return _META


# ---------------------------------------------------------------- bass program

# blob1 bf16-column layout: [wkv 8x128 | wq 8x64 | idb 128 | bias(f32) 4]
_B1_WKV = 0
_B1_WQ = 1024
_B1_IDB = 1536
_B1_BIAS = 1664
_B1_COLS = 1668


def _build():
    import concourse.mybir as mybir
    import concourse.tile as tile
    from concourse import bacc

    f32 = mybir.dt.float32
    bf16 = mybir.dt.bfloat16

    OFF_C, ENDS = _mask_meta()

    nc = bacc.Bacc("TRN2", target_bir_lowering=False, debug=False, num_devices=8)

    xh = nc.dram_tensor("xh", [NT, 128, 8, 512], bf16, kind="ExternalInput").ap()
    blob1 = nc.dram_tensor("blob1", [128, _B1_COLS], bf16, kind="ExternalInput").ap()
    maskd = nc.dram_tensor("maskd", [128, 8 * 512], bf16, kind="ExternalInput").ap()
    wobo = nc.dram_tensor("wobo", [65, 1024], bf16, kind="ExternalInput").ap()
    y = nc.dram_tensor("y", [2048, 1024], bf16, kind="ExternalOutput").ap()

    Exp = mybir.ActivationFunctionType.Exp
    mult = mybir.AluOpType.mult
    add_op = mybir.AluOpType.add
    scale = 1.0 / math.sqrt(D)

    with tile.TileContext(nc) as tc:
        with (
            tc.tile_pool(name="consts", bufs=1) as consts,
            tc.tile_pool(name="xp", bufs=3) as xpool,
            tc.tile_pool(name="ep", bufs=6) as epool,
            tc.tile_pool(name="osp", bufs=4) as ospool,
            tc.tile_pool(name="yb", bufs=2) as ypool,
            tc.tile_pool(name="pkv", bufs=1, space="PSUM") as pkv,
            tc.tile_pool(name="pq", bufs=1, space="PSUM") as pq,
            tc.tile_pool(name="po", bufs=2, space="PSUM") as po,
            tc.tile_pool(name="ps", bufs=2, space="PSUM") as ps,
        ):
            # ---- constants. blob1 on the ACT DMA stream so x0 (SP stream)
            # transfers concurrently.
            blob1_sb = consts.tile([128, _B1_COLS], bf16)
            nc.scalar.dma_start(blob1_sb[:], blob1[:])
            wkv_v = blob1_sb[:, _B1_WKV : _B1_WKV + 1024].rearrange(
                "p (c f) -> p c f", c=8
            )
            wq_v = blob1_sb[:, _B1_WQ : _B1_WQ + 512].rearrange("p (c f) -> p c f", c=8)
            idb_v = blob1_sb[:, _B1_IDB : _B1_IDB + 128]
            bias_v = blob1_sb[:, _B1_BIAS : _B1_BIAS + 4].bitcast(f32)

            kvt_sb = consts.tile([128, N], bf16)  # rows 0:64 KT, 64:128 VT
            qt_sb = consts.tile([H, 2048], bf16)
            v65_sb = consts.tile([128, 32 * 65], bf16)
            nc.vector.memset(
                v65_sb[:].rearrange("p (t c) -> p t c", c=65)[:, :, 64:65], 1.0
            )
            mask_sb = consts.tile([128, 8 * 512], bf16)
            wobo_sb = consts.tile([65, 1024], bf16)
            ot_sb = consts.tile([65, 2048], bf16)
            recip_sb = consts.tile([128, 16], f32)
            scratch_sb = consts.tile([1, 8], f32)

            # prewarm the ACT exp table while DMAs stream (ACT stream order:
            # blob1 -> prewarm -> x1 -> masks -> wobo -> x3)
            nc.scalar.activation(
                scratch_sb[:], idb_v[0:1, 0:8], Exp, bias=0.0, scale=1.0
            )

            # PE p-state warmup on blob1 data during the x0 fill
            warm = ps.tile([128, 1024], f32, tag="s")
            for _ in range(3):
                nc.tensor.matmul(
                    warm[:, 0:512],
                    idb_v[:],
                    blob1_sb[:, 0:512],
                    start=True,
                    stop=True,
                )

            # ---------------- stage A: projections for ntile n
            def emit_a(n):
                xn = xpool.tile([128, 4096], bf16, tag="x")
                xnv = xn[:].rearrange("p (c f) -> p c f", c=8)
                if n == 0:
                    # split first load so PE starts sooner
                    nc.sync.dma_start(xnv[:, 0:4], xh[0][:, 0:4])
                    nc.sync.dma_start(xnv[:, 4:8], xh[0][:, 4:8])
                elif n in (1, 3):
                    nc.scalar.dma_start(xnv[:], xh[n])
                else:
                    nc.sync.dma_start(xnv[:], xh[n])
                kvp = pkv.tile([128, 512], f32, tag="kv")
                qp = pq.tile([64, 256], f32, tag="q")
                for c in range(8):
                    nc.tensor.matmul(
                        kvp[:],
                        wkv_v[:, c],
                        xn[:, 512 * c : 512 * c + 512],
                        start=(c == 0),
                        stop=(c == 7),
                    )
                    nc.tensor.matmul(
                        qp[:],
                        wq_v[:, c],
                        xn[:, 512 * c : 512 * c + 256],
                        start=(c == 0),
                        stop=(c == 7),
                    )
                nc.vector.tensor_scalar(
                    out=kvt_sb[:, 512 * n : 512 * (n + 1)],
                    in0=kvp[:],
                    scalar1=bias_v[:, 0:1],
                    scalar2=None,
                    op0=add_op,
                )
                nc.vector.tensor_scalar(
                    out=qt_sb[:, 256 * n : 256 * (n + 1)],
                    in0=qp[:],
                    scalar1=bias_v[0:64, 1:2],
                    scalar2=None,
                    op0=add_op,
                )
                vp = pkv.tile([128, 256], bf16, tag="kv")
                for i, t in enumerate(range(4 * n, 4 * n + 4)):
                    nc.tensor.transpose(
                        vp[:, 64 * i : 64 * (i + 1)],
                        kvt_sb[64:128, 128 * t : 128 * (t + 1)],
                        idb_v[64:128, 64:128],
                    )
                nc.vector.tensor_copy(
                    v65_sb[:].rearrange("p (t c) -> p t c", c=65)[
                        :, 4 * n : 4 * n + 4, 0:64
                    ],
                    vp[:].rearrange("p (t c) -> p t c", c=64),
                )

            # ---------------- per-chunk finish: OT transpose + D + y copy.
            # pending entries: (j, c, osb, ys)
            pending = []

            def emit_chunk_d(task):
                j, c, osb, ys = task
                i = 4 * j + c
                pot = pq.tile([65, 128], bf16, tag="q")
                nc.tensor.transpose(pot[:], osb[:], idb_v[:])
                nc.gpsimd.tensor_copy(ot_sb[:, 128 * i : 128 * (i + 1)], pot[:])
                yp = ps.tile([128, 1024], f32, tag="s")
                for d in range(2):
                    nc.tensor.matmul(
                        yp[:, 512 * d : 512 * (d + 1)],
                        ot_sb[:, 128 * i : 128 * (i + 1)],
                        wobo_sb[:, 512 * d : 512 * (d + 1)],
                        start=True,
                        stop=True,
                    )
                eng = nc.vector if i % 2 == 0 else nc.gpsimd
                eng.tensor_copy(ys[:, 1024 * c : 1024 * (c + 1)], yp[:])
                if c == 3:
                    nc.gpsimd.dma_start(
                        y[512 * j : 512 * (j + 1), :].rearrange(
                            "(t p) d -> p t d", p=128
                        ),
                        ys[:].rearrange("p (t d) -> p t d", t=4),
                    )

            # ---------------- stage B + C for slot j
            def emit_slot(j):
                nk = 8 * (j + 1)
                o_ps = po.tile([128, 4 * 65], f32, tag="o")
                ys = ypool.tile([128, 4096], bf16, tag="ys")
                # PSUM accumulation groups are bank-granular (2KB): all four
                # 65-col chunk accumulators share one bank, so start/stop go
                # on the first/last live matmul of the whole slot (start's
                # pending-zero covers the full bank).
                def live(t, c):
                    rr = t - 8 * j
                    return rr < 0 or OFF_C[rr] < 128 * (c + 1)

                lives = [(t, c) for t in range(nk) for c in range(4) if live(t, c)]
                first_tc, last_tc = lives[0], lives[-1]

                def emit_ct(t0, et, offs):
                    for h in range(2):
                        t = t0 + h
                        for c in range(4):
                            if not live(t, c):
                                continue  # chunk fully causally dead
                            nc.tensor.matmul(
                                o_ps[:, 65 * c : 65 * c + 65],
                                et[:, 512 * h + 128 * c : 512 * h + 128 * (c + 1)],
                                v65_sb[:, 65 * t : 65 * (t + 1)],
                                start=((t, c) == first_tc),
                                stop=((t, c) == last_tc),
                            )

                prevs = []
                for t0 in range(0, nk, 2):
                    if pending and t0 % 4 == 2:
                        emit_chunk_d(pending.pop(0))
                    sp = ps.tile([128, 1024], f32, tag="s")
                    offs = []
                    for h in range(2):
                        t = t0 + h
                        rr = t - 8 * j
                        off = 0 if rr < 0 else OFF_C[rr]
                        offs.append(off)
                        nc.tensor.matmul(
                            sp[:, 512 * h + off : 512 * (h + 1)],
                            kvt_sb[0:64, 128 * t : 128 * (t + 1)],
                            qt_sb[:, 512 * j + off : 512 * (j + 1)],
                            start=True,
                            stop=(rr < 0),
                        )
                        if rr >= 0:
                            end = ENDS[rr]
                            nc.tensor.matmul(
                                sp[:, 512 * h + off : 512 * h + end],
                                idb_v[:],
                                mask_sb[:, 512 * rr + off : 512 * rr + end],
                                start=False,
                                stop=True,
                            )
                    et = epool.tile([128, 1024], bf16, tag="e")
                    if offs == [0, 0]:
                        nc.scalar.activation(et[:], sp[:], Exp, bias=0.0, scale=scale)
                    elif offs[0] == offs[1]:
                        o = offs[0]
                        nc.scalar.activation(
                            et[:].rearrange("p (b f) -> p b f", b=2)[:, :, o:512],
                            sp[:].rearrange("p (b f) -> p b f", b=2)[:, :, o:512],
                            Exp,
                            bias=0.0,
                            scale=scale,
                        )
                    else:
                        for h in range(2):
                            o = 512 * h + offs[h]
                            nc.scalar.activation(
                                et[:, o : 512 * (h + 1)],
                                sp[:, o : 512 * (h + 1)],
                                Exp,
                                bias=0.0,
                                scale=scale,
                            )
                    prevs.append((t0, et, offs))
                    if len(prevs) > 2:
                        emit_ct(*prevs.pop(0))
                for p in prevs:
                    emit_ct(*p)
                # slot finish: denominators + normalize/evacuate O to bf16
                for c in range(4):
                    i = 4 * j + c
                    nc.vector.reciprocal(
                        recip_sb[:, i : i + 1], o_ps[:, 65 * c + 64 : 65 * c + 65]
                    )
                    osb = ospool.tile([128, 65], bf16, tag="osb")
                    nc.vector.tensor_scalar(
                        out=osb[:],
                        in0=o_ps[:, 65 * c : 65 * c + 65],
                        scalar1=recip_sb[:, i : i + 1],
                        scalar2=None,
                        op0=mult,
                    )
                    pending.append((j, c, osb, ys))

            for n in range(NT):
                emit_a(n)
                if n == 1:
                    # second consts wave on the ACT stream (after x1)
                    nc.scalar.dma_start(
                        mask_sb[:].rearrange("p (m f) -> p m f", m=8),
                        maskd.rearrange("p (m f) -> p m f", m=8),
                    )
                    nc.scalar.dma_start(wobo_sb[:], wobo[:])
                if n % 2 == 1:
                    emit_slot((n - 1) // 2)

            for task in pending:
                emit_chunk_d(task)

    nc.compile()
    return nc


def _get_prog():
    global _PROG
    if _PROG is None:
        _PROG = _build()
    return _PROG


# ---------------------------------------------------------------- host inputs


def _xh(xb, korder):
    """[ntile, partition, chunk, 512] bf16 layout of x[b][korder].T."""
    import ml_dtypes

    xt = xb[korder].T  # [1024, 4096]
    return np.ascontiguousarray(
        xt.reshape(8, 128, 8, 512).transpose(2, 1, 0, 3).astype(ml_dtypes.bfloat16)
    )


def _blob1(Wq, bq, Wk, bk, Wov, bov):
    import ml_dtypes

    blob = np.zeros((128, _B1_COLS), dtype=ml_dtypes.bfloat16)
    wkv_t = np.concatenate([Wk, Wov], axis=0).T.astype(ml_dtypes.bfloat16)  # [1024,128]
    blob[:, _B1_WKV : _B1_WKV + 1024] = (
        wkv_t.reshape(8, 128, 128).transpose(1, 0, 2).reshape(128, 1024)
    )
    wq_t = Wq.T.astype(ml_dtypes.bfloat16)  # [1024, 64]
    blob[:, _B1_WQ : _B1_WQ + 512] = (
        wq_t.reshape(8, 128, 64).transpose(1, 0, 2).reshape(128, 512)
    )
    blob[:, _B1_IDB : _B1_IDB + 128] = np.eye(128, dtype=ml_dtypes.bfloat16)
    biases = np.zeros((128, 2), dtype=np.float32)
    biases[:, 0] = np.concatenate([bk, bov])
    biases[0:64, 1] = bq
    blob[:, _B1_BIAS : _B1_BIAS + 4] = biases.view(np.uint16).view(ml_dtypes.bfloat16)
    return blob


def _in_map(x, Wq, bq, Wk, bk, Wov, bov, Wo, bo, core):
    import ml_dtypes

    b, s = divmod(core, 2)
    maskd = (
        _masks(s)
        .astype(ml_dtypes.bfloat16)
        .transpose(1, 0, 2)
        .reshape(128, 8 * 512)
    )
    return {
        "xh": _xh(x[b], _key_order(s)),
        "blob1": _blob1(Wq, bq, Wk, bk, Wov, bov),
        "maskd": np.ascontiguousarray(maskd),
        "wobo": np.concatenate([Wo.T, bo[None, :]], axis=0).astype(ml_dtypes.bfloat16),
    }


# ---------------------------------------------------------------- entry point


def kernel(x, Wq, bq, Wk, bk, Wov, bov, Wo, bo, _trace=False):
    from concourse import bass_utils

    x = np.ascontiguousarray(np.asarray(x, dtype=np.float32))
    args = [np.asarray(a, dtype=np.float32) for a in (Wq, bq, Wk, bk, Wov, bov, Wo, bo)]

    nc = _get_prog()
    in_maps = [_in_map(x, *args, core) for core in range(8)]

    res = bass_utils.run_bass_kernel_spmd(
        nc, in_maps, core_ids=list(range(8)), trace=_trace
    )

    y = np.empty((B, N, D), dtype=np.float32)
    for core in range(8):
        b, s = divmod(core, 2)
        yc = np.asarray(res.results[core]["y"]).astype(np.float32)
        y[b].reshape(64, CH, D)[s::2] = yc.reshape(32, CH, D)
    return y


# revision 14
# speedup vs baseline: 1.0965x; 1.0768x over previous
"""Trainium2 Bass kernel for nn_AttentionHead (B=4, n_ctx=4096, d_model=1024,
d_hidden=64, causal, scale=1/sqrt(d_model)).

Sharding: 8 cores = 4 batches x 2 balanced causal shards. Core (b, s) handles
the 2048 query rows in 64-row chunks with chunk%2 == s. Keys/x-columns are
permuted per core (my-parity chunks first within each 512-key ntile) so that
every core runs the IDENTICAL SPMD program:

  - slot j (0..3) = 512 queries = my chunks of ntiles 2j, 2j+1
  - slot j attends k-tiles t = 0..8(j+1)-1 (128 permuted keys each)
  - k-tiles t < 8j are fully open; t = 8j + r (r in 0..7) get an additive
    causal mask that depends only on (r, s) -> 8 mask tiles per core, sent
    as data.

v2 design notes (cost model: matmul cost = moving-dim rows only; DMA
transfers from different issuing engines overlap; same-engine serialize):

  A: KT/VT = [Wk;Wov] @ xT fused (PSUM-accum over 8 d_model chunks), Q
     likewise; PSUM->SBUF copies add biases and downcast to bf16 (bf16
     matmuls are full rate at any N, unlike f32r's N>=256). V transposed
     to natural [k,64] bf16 layout via PE transpose; v65 keeps an
     appended ones column (DVE memset) so E^T@[V|1] also yields the
     softmax denominator.
  B: S^T[k,q] = KT_tile^T @ QT_slot -> PSUM pair tile; additive mask via
     identity matmul for diagonal tiles; exp((S+M)/32) on ACT -> bf16 E.
  C (transposed vs v1): O[q,65] += E_chunk^T @ V65_tile, i.e. E is the
     stationary operand and the 65-wide V65 is moving: 65 rows/chunk-tile
     instead of 512/tile -- less than half the PE cost of v1's C. Col 64
     accumulates the denominator per q-partition.
  D: per 128-q chunk: recip = 1/O[:,64] (DVE), normalize-copy
     O*recip -> bf16 (fused into the mandatory PSUM evacuation; makes
     col 64 exactly 1.0), PE-transpose to OT[65,128], Pool-copy to SBUF,
     then y = OT^T @ [Wo^T; bo] -- the 1.0 row adds bo exactly, so no
     per-element recip multiply is needed after the matmul. y is copied
     to bf16 (DVE/Pool alternating) and stored per-slot.

DMA: three independent streams. SP carries x ntiles (x0 split in half so
PE starts ~1.5us earlier); ACT carries the consts blob + x1/x3 + masks +
wobo (interleaved so each lands just before first use); Pool carries the
4 per-slot bf16 y stores (SWDGE). y is written bf16 and upcast on host
(adds ~0.2% fro error vs the 2e-2 budget). A few warmup matmuls on the
consts blob ramp the PE p-state during the DMA fill.
"""

import math

import numpy as np

D = 1024
H = 64
N = 4096
B = 4
CH = 64  # query chunk size (rows)
NT = 8  # ntiles of 512 keys
NEG = -1e10

_PROG = None  # cached compiled program
_META = None  # cached mask offsets/ends


# ---------------------------------------------------------------- host layout


def _key_order(s: int) -> np.ndarray:
    order = []
    for n in range(NT):
        mine = [8 * n + t for t in range(8) if t % 2 == s]
        theirs = [8 * n + t for t in range(8) if t % 2 != s]
        for c in mine + theirs:
            order.extend(range(CH * c, CH * c + CH))
    return np.array(order)


def _masks(s: int) -> np.ndarray:
    ko = _key_order(s)
    qo = np.array([CH * c + i for c in range(s, 64, 2) for i in range(CH)])
    m = np.zeros((8, 128, 512), dtype=np.float32)
    for r in range(8):
        keys = ko[128 * r : 128 * (r + 1)]
        qs = qo[0:512]
        m[r] = np.where(keys[:, None] <= qs[None, :], 0.0, NEG)
    return m


def _mask_meta():
    """Per masked-tile r (min/max over both parities so the shared program is
    valid for either): OFF_C = 128-floored fully-dead q-prefix, END = end of
    the nonzero mask band."""
    global _META
    if _META is not None:
        return _META
    offs = []
    ends = []
    ms = [_masks(0), _masks(1)]
    for r in range(8):
        offr, endr = [], []
        for s in (0, 1):
            dead = ms[s][r] != 0.0
            colall = dead.all(axis=0)
            off = 0
            while off < 512 and colall[off]:
                off += 1
            anyd = dead.any(axis=1).any()
            cols = np.nonzero(dead.any(axis=0))[0]
            end = int(cols.max()) + 1 if cols.size else 0
            offr.append(off)
            endr.append(end)
        offs.append((min(offr) // 128) * 128)
        ends.append(max(endr))
    _META = (offs, ends)
    return _META


# ---------------------------------------------------------------- bass program

# blobw bf16-column layout: [idb 128 | bias(f32) 4]  (tiny, lands first so
# the PE warmup can start during the x0 fill)
_BW_IDB = 0
_BW_BIAS = 128
_BW_COLS = 132
# blob1 bf16-column layout: [wkv 8x128 | wq 8x64]
_B1_WKV = 0
_B1_WQ = 1024
_B1_COLS = 1536


def _build():
    import concourse.mybir as mybir
    import concourse.tile as tile
    from concourse import bacc

    f32 = mybir.dt.float32
    bf16 = mybir.dt.bfloat16

    OFF_C, ENDS = _mask_meta()

    nc = bacc.Bacc("TRN2", target_bir_lowering=False, debug=False, num_devices=8)

    xh = nc.dram_tensor("xh", [NT, 128, 8, 512], bf16, kind="ExternalInput").ap()
    blobw = nc.dram_tensor("blobw", [128, _BW_COLS], bf16, kind="ExternalInput").ap()
    blob1 = nc.dram_tensor("blob1", [128, _B1_COLS], bf16, kind="ExternalInput").ap()
    maskd = nc.dram_tensor("maskd", [128, 8 * 512], bf16, kind="ExternalInput").ap()
    wobo = nc.dram_tensor("wobo", [65, 1024], bf16, kind="ExternalInput").ap()
    y = nc.dram_tensor("y", [2048, 1024], bf16, kind="ExternalOutput").ap()

    Exp = mybir.ActivationFunctionType.Exp
    mult = mybir.AluOpType.mult
    add_op = mybir.AluOpType.add
    scale = 1.0 / math.sqrt(D)

    with tile.TileContext(nc) as tc:
        with (
            tc.tile_pool(name="consts", bufs=1) as consts,
            tc.tile_pool(name="xp", bufs=3) as xpool,
            tc.tile_pool(name="ep", bufs=6) as epool,
            tc.tile_pool(name="osp", bufs=4) as ospool,
            tc.tile_pool(name="yb", bufs=2) as ypool,
            tc.tile_pool(name="pkv", bufs=1, space="PSUM") as pkv,
            tc.tile_pool(name="pq", bufs=1, space="PSUM") as pq,
            tc.tile_pool(name="po", bufs=2, space="PSUM") as po,
            tc.tile_pool(name="ps", bufs=2, space="PSUM") as ps,
        ):
            # ---- constants. blobw/blob1 on the ACT DMA stream so x0 (SP
            # stream) transfers concurrently.
            blobw_sb = consts.tile([128, _BW_COLS], bf16)
            nc.scalar.dma_start(blobw_sb[:], blobw[:])
            blob1_sb = consts.tile([128, _B1_COLS], bf16)
            nc.scalar.dma_start(blob1_sb[:], blob1[:])
            wkv_v = blob1_sb[:, _B1_WKV : _B1_WKV + 1024].rearrange(
                "p (c f) -> p c f", c=8
            )
            wq_v = blob1_sb[:, _B1_WQ : _B1_WQ + 512].rearrange("p (c f) -> p c f", c=8)
            idb_v = blobw_sb[:, _BW_IDB : _BW_IDB + 128]
            bias_v = blobw_sb[:, _BW_BIAS : _BW_BIAS + 4].bitcast(f32)

            kvt_sb = consts.tile([128, N], bf16)  # rows 0:64 KT, 64:128 VT
            qt_sb = consts.tile([H, 2048], bf16)
            v65_sb = consts.tile([128, 32 * 65], bf16)
            nc.vector.memset(
                v65_sb[:].rearrange("p (t c) -> p t c", c=65)[:, :, 64:65], 1.0
            )
            mask_sb = consts.tile([128, 8 * 512], bf16)
            wobo_sb = consts.tile([65, 1024], bf16)
            ot_sb = consts.tile([65, 2048], bf16)
            recip_sb = consts.tile([128, 16], f32)
            scratch_sb = consts.tile([1, 8], f32)

            # prewarm the ACT exp table while DMAs stream (ACT stream order:
            # blob1 -> prewarm -> x1 -> masks -> wobo -> x3)
            nc.scalar.activation(
                scratch_sb[:], idb_v[0:1, 0:8], Exp, bias=0.0, scale=1.0
            )

            # PE p-state warmup on blobw data during the x0 fill
            warm = ps.tile([128, 1024], f32, tag="s")
            for _ in range(8):
                nc.tensor.matmul(
                    warm[:, 0:128],
                    idb_v[:],
                    idb_v[:],
                    start=True,
                    stop=True,
                )

            # ---------------- stage A: projections for ntile n
            def emit_a(n):
                xn = xpool.tile([128, 4096], bf16, tag="x")
                xnv = xn[:].rearrange("p (c f) -> p c f", c=8)
                if n == 0:
                    # split first load so PE starts sooner
                    nc.sync.dma_start(xnv[:, 0:4], xh[0][:, 0:4])
                    nc.sync.dma_start(xnv[:, 4:8], xh[0][:, 4:8])
                elif n in (1, 3):
                    nc.scalar.dma_start(xnv[:], xh[n])
                else:
                    nc.sync.dma_start(xnv[:], xh[n])
                # KV first, then the kvt evacuation (DVE) hides under the Q
                # matmuls, and the qt evacuation under the V transposes.
                kvp = pkv.tile([128, 512], f32, tag="kv")
                qp = pq.tile([64, 256], f32, tag="q")
                for c in range(8):
                    nc.tensor.matmul(
                        kvp[:],
                        wkv_v[:, c],
                        xn[:, 512 * c : 512 * c + 512],
                        start=(c == 0),
                        stop=(c == 7),
                    )
                nc.vector.tensor_scalar(
                    out=kvt_sb[:, 512 * n : 512 * (n + 1)],
                    in0=kvp[:],
                    scalar1=bias_v[:, 0:1],
                    scalar2=None,
                    op0=add_op,
                )
                for c in range(8):
                    nc.tensor.matmul(
                        qp[:],
                        wq_v[:, c],
                        xn[:, 512 * c : 512 * c + 256],
                        start=(c == 0),
                        stop=(c == 7),
                    )
                nc.vector.tensor_scalar(
                    out=qt_sb[:, 256 * n : 256 * (n + 1)],
                    in0=qp[:],
                    scalar1=bias_v[0:64, 1:2],
                    scalar2=None,
                    op0=add_op,
                )
                vp = pkv.tile([128, 256], bf16, tag="kv")
                for i, t in enumerate(range(4 * n, 4 * n + 4)):
                    nc.tensor.transpose(
                        vp[:, 64 * i : 64 * (i + 1)],
                        kvt_sb[64:128, 128 * t : 128 * (t + 1)],
                        idb_v[64:128, 64:128],
                    )
                nc.vector.tensor_copy(
                    v65_sb[:].rearrange("p (t c) -> p t c", c=65)[
                        :, 4 * n : 4 * n + 4, 0:64
                    ],
                    vp[:].rearrange("p (t c) -> p t c", c=64),
                )

            # ---------------- per-chunk finish in two phases so the Pool
            # ot copy of chunk c overlaps PE work of the previous phase-2:
            #   phase 1: OT transpose (PE) + ot_sb copy (Pool)
            #   phase 2: D matmuls (PE) + y copy (Pool) + per-slot store
            phase1 = []
            phase2 = []

            def emit_phase1(task):
                j, c, osb, ys = task
                i = 4 * j + c
                pot = pq.tile([65, 128], bf16, tag="q")
                nc.tensor.transpose(pot[:], osb[:], idb_v[:])
                nc.gpsimd.tensor_copy(ot_sb[:, 128 * i : 128 * (i + 1)], pot[:])
                phase2.append(task)

            def emit_phase2(task, split_store=False):
                j, c, _, ys = task
                i = 4 * j + c
                yp = ps.tile([128, 1024], f32, tag="s")
                for d in range(2):
                    nc.tensor.matmul(
                        yp[:, 512 * d : 512 * (d + 1)],
                        ot_sb[:, 128 * i : 128 * (i + 1)],
                        wobo_sb[:, 512 * d : 512 * (d + 1)],
                        start=True,
                        stop=True,
                    )
                nc.gpsimd.tensor_copy(ys[:, 1024 * c : 1024 * (c + 1)], yp[:])
                if split_store:
                    nc.gpsimd.dma_start(
                        y[512 * j + 128 * c : 512 * j + 128 * (c + 1), :],
                        ys[:, 1024 * c : 1024 * (c + 1)],
                    )
                elif c == 3:
                    nc.gpsimd.dma_start(
                        y[512 * j : 512 * (j + 1), :].rearrange(
                            "(t p) d -> p t d", p=128
                        ),
                        ys[:].rearrange("p (t d) -> p t d", t=4),
                    )

            def pop_pending():
                # phase 1 of the next chunk before phase 2 of the previous
                if phase1:
                    emit_phase1(phase1.pop(0))
                elif phase2:
                    emit_phase2(phase2.pop(0))

            # ---------------- stage B + C for slot j
            def emit_slot(j):
                nk = 8 * (j + 1)
                o_ps = po.tile([128, 4 * 65], f32, tag="o")
                ys = ypool.tile([128, 4096], bf16, tag="ys")

                def live(t, c):
                    rr = t - 8 * j
                    return rr < 0 or OFF_C[rr] < 128 * (c + 1)

                # open tiles pair consecutively; masked tiles are paired so
                # both halves share the same causal offset, letting a single
                # strided exp cover the pair
                pairs = [(t, t + 1) for t in range(0, 8 * j, 2)] + [
                    (8 * j + a, 8 * j + b)
                    for a, b in ((0, 2), (1, 3), (4, 6), (5, 7))
                ]
                # PSUM accumulation groups are bank-granular (2KB): all four
                # 65-col chunk accumulators share one bank, so start/stop go
                # on the first/last live matmul in emission order (start's
                # pending-zero covers the full bank).
                lives = [
                    (t, c) for pr in pairs for t in pr for c in range(4) if live(t, c)
                ]
                first_tc, last_tc = lives[0], lives[-1]

                def emit_ct(pr, et, offs):
                    for h in range(2):
                        t = pr[h]
                        for c in range(4):
                            if not live(t, c):
                                continue  # chunk fully causally dead
                            nc.tensor.matmul(
                                o_ps[:, 65 * c : 65 * c + 65],
                                et[:, 512 * h + 128 * c : 512 * h + 128 * (c + 1)],
                                v65_sb[:, 65 * t : 65 * (t + 1)],
                                start=((t, c) == first_tc),
                                stop=((t, c) == last_tc),
                            )

                prevs = []
                for pr in pairs:
                    pop_pending()
                    sp = ps.tile([128, 1024], f32, tag="s")
                    offs = []
                    for h in range(2):
                        t = pr[h]
                        rr = t - 8 * j
                        off = 0 if rr < 0 else OFF_C[rr]
                        offs.append(off)
                        nc.tensor.matmul(
                            sp[:, 512 * h + off : 512 * (h + 1)],
                            kvt_sb[0:64, 128 * t : 128 * (t + 1)],
                            qt_sb[:, 512 * j + off : 512 * (j + 1)],
                            start=True,
                            stop=(rr < 0),
                        )
                        if rr >= 0:
                            end = ENDS[rr]
                            nc.tensor.matmul(
                                sp[:, 512 * h + off : 512 * h + end],
                                idb_v[:],
                                mask_sb[:, 512 * rr + off : 512 * rr + end],
                                start=False,
                                stop=True,
                            )
                    et = epool.tile([128, 1024], bf16, tag="e")
                    if offs == [0, 0]:
                        nc.scalar.activation(et[:], sp[:], Exp, bias=0.0, scale=scale)
                    elif offs[0] == offs[1]:
                        o = offs[0]
                        nc.scalar.activation(
                            et[:].rearrange("p (b f) -> p b f", b=2)[:, :, o:512],
                            sp[:].rearrange("p (b f) -> p b f", b=2)[:, :, o:512],
                            Exp,
                            bias=0.0,
                            scale=scale,
                        )
                    else:
                        for h in range(2):
                            o = 512 * h + offs[h]
                            nc.scalar.activation(
                                et[:, o : 512 * (h + 1)],
                                sp[:, o : 512 * (h + 1)],
                                Exp,
                                bias=0.0,
                                scale=scale,
                            )
                    prevs.append((pr, et, offs))
                    if len(prevs) > 2:
                        emit_ct(*prevs.pop(0))
                for p in prevs:
                    emit_ct(*p)
                # slot finish: denominators + normalize/evacuate O to bf16
                for c in range(4):
                    i = 4 * j + c
                    nc.vector.reciprocal(
                        recip_sb[:, i : i + 1], o_ps[:, 65 * c + 64 : 65 * c + 65]
                    )
                    osb = ospool.tile([128, 65], bf16, tag="osb")
                    nc.vector.tensor_scalar(
                        out=osb[:],
                        in0=o_ps[:, 65 * c : 65 * c + 65],
                        scalar1=recip_sb[:, i : i + 1],
                        scalar2=None,
                        op0=mult,
                    )
                    phase1.append((j, c, osb, ys))

            for n in range(NT):
                emit_a(n)
                if n == 1:
                    # second consts wave on the ACT stream (after x1)
                    nc.scalar.dma_start(
                        mask_sb[:].rearrange("p (m f) -> p m f", m=8),
                        maskd.rearrange("p (m f) -> p m f", m=8),
                    )
                    nc.scalar.dma_start(wobo_sb[:], wobo[:])
                if n % 2 == 1:
                    emit_slot((n - 1) // 2)

            # tail: drain slot 3 with per-chunk stores so the final store is
            # a quarter-slot, shortening the serial tail
            for task in phase1:
                emit_phase1(task)
            phase1 = []
            for task in phase2:
                emit_phase2(task, split_store=True)

    nc.compile()
    return nc


def _get_prog():
    global _PROG
    if _PROG is None:
        _PROG = _build()
    return _PROG


# ---------------------------------------------------------------- host inputs


def _xh(xb, korder):
    """[ntile, partition, chunk, 512] bf16 layout of x[b][korder].T."""
    import ml_dtypes

    xt = xb[korder].T  # [1024, 4096]
    return np.ascontiguousarray(
        xt.reshape(8, 128, 8, 512).transpose(2, 1, 0, 3).astype(ml_dtypes.bfloat16)
    )


def _blobw(bq, bk, bov):
    import ml_dtypes

    blob = np.zeros((128, _BW_COLS), dtype=ml_dtypes.bfloat16)
    blob[:, _BW_IDB : _BW_IDB + 128] = np.eye(128, dtype=ml_dtypes.bfloat16)
    biases = np.zeros((128, 2), dtype=np.float32)
    biases[:, 0] = np.concatenate([bk, bov])
    biases[0:64, 1] = bq
    blob[:, _BW_BIAS : _BW_BIAS + 4] = biases.view(np.uint16).view(ml_dtypes.bfloat16)
    return blob


def _blob1(Wq, Wk, Wov):
    import ml_dtypes

    blob = np.zeros((128, _B1_COLS), dtype=ml_dtypes.bfloat16)
    wkv_t = np.concatenate([Wk, Wov], axis=0).T.astype(ml_dtypes.bfloat16)  # [1024,128]
    blob[:, _B1_WKV : _B1_WKV + 1024] = (
        wkv_t.reshape(8, 128, 128).transpose(1, 0, 2).reshape(128, 1024)
    )
    wq_t = Wq.T.astype(ml_dtypes.bfloat16)  # [1024, 64]
    blob[:, _B1_WQ : _B1_WQ + 512] = (
        wq_t.reshape(8, 128, 64).transpose(1, 0, 2).reshape(128, 512)
    )
    return blob


def _in_map(x, Wq, bq, Wk, bk, Wov, bov, Wo, bo, core):
    import ml_dtypes

    b, s = divmod(core, 2)
    maskd = (
        _masks(s)
        .astype(ml_dtypes.bfloat16)
        .transpose(1, 0, 2)
        .reshape(128, 8 * 512)
    )
    return {
        "xh": _xh(x[b], _key_order(s)),
        "blobw": _blobw(bq, bk, bov),
        "blob1": _blob1(Wq, Wk, Wov),
        "maskd": np.ascontiguousarray(maskd),
        "wobo": np.concatenate([Wo.T, bo[None, :]], axis=0).astype(ml_dtypes.bfloat16),
    }


# ---------------------------------------------------------------- entry point


def kernel(x, Wq, bq, Wk, bk, Wov, bov, Wo, bo, _trace=False):
    from concourse import bass_utils

    x = np.ascontiguousarray(np.asarray(x, dtype=np.float32))
    args = [np.asarray(a, dtype=np.float32) for a in (Wq, bq, Wk, bk, Wov, bov, Wo, bo)]

    nc = _get_prog()
    in_maps = [_in_map(x, *args, core) for core in range(8)]

    res = bass_utils.run_bass_kernel_spmd(
        nc, in_maps, core_ids=list(range(8)), trace=_trace
    )

    y = np.empty((B, N, D), dtype=np.float32)
    for core in range(8):
        b, s = divmod(core, 2)
        yc = np.asarray(res.results[core]["y"]).astype(np.float32)
        y[b].reshape(64, CH, D)[s::2] = yc.reshape(32, CH, D)
    return y


# revision 19
# speedup vs baseline: 1.1042x; 1.0071x over previous
"""Trainium2 Bass kernel for nn_AttentionHead (B=4, n_ctx=4096, d_model=1024,
d_hidden=64, causal, scale=1/sqrt(d_model)).

Sharding: 8 cores = 4 batches x 2 balanced causal shards. Core (b, s) handles
the 2048 query rows in 64-row chunks with chunk%2 == s. Keys/x-columns are
permuted per core (my-parity chunks first within each 512-key ntile) so that
every core runs the IDENTICAL SPMD program:

  - slot j (0..3) = 512 queries = my chunks of ntiles 2j, 2j+1
  - slot j attends k-tiles t = 0..8(j+1)-1 (128 permuted keys each)
  - k-tiles t < 8j are fully open; t = 8j + r (r in 0..7) get an additive
    causal mask that depends only on (r, s) -> 8 mask tiles per core, sent
    as data.

v2 design notes (cost model: matmul cost = moving-dim rows only; DMA
transfers from different issuing engines overlap; same-engine serialize):

  A: KT/VT = [Wk;Wov] @ xT fused (PSUM-accum over 8 d_model chunks), Q
     likewise; PSUM->SBUF copies add biases and downcast to bf16 (bf16
     matmuls are full rate at any N, unlike f32r's N>=256). V transposed
     to natural [k,64] bf16 layout via PE transpose; v65 keeps an
     appended ones column (DVE memset) so E^T@[V|1] also yields the
     softmax denominator.
  B: S^T[k,q] = KT_tile^T @ QT_slot -> PSUM pair tile; additive mask via
     identity matmul for diagonal tiles; exp((S+M)/32) on ACT -> bf16 E.
  C (transposed vs v1): O[q,65] += E_chunk^T @ V65_tile, i.e. E is the
     stationary operand and the 65-wide V65 is moving: 65 rows/chunk-tile
     instead of 512/tile -- less than half the PE cost of v1's C. Col 64
     accumulates the denominator per q-partition.
  D: per 128-q chunk: recip = 1/O[:,64] (DVE), normalize-copy
     O*recip -> bf16 (fused into the mandatory PSUM evacuation; makes
     col 64 exactly 1.0), PE-transpose to OT[65,128], Pool-copy to SBUF,
     then y = OT^T @ [Wo^T; bo] -- the 1.0 row adds bo exactly, so no
     per-element recip multiply is needed after the matmul. y is copied
     to bf16 (DVE/Pool alternating) and stored per-slot.

DMA: three independent streams. SP carries x ntiles (x0 split in half so
PE starts ~1.5us earlier); ACT carries the consts blob + x1/x3 + masks +
wobo (interleaved so each lands just before first use); Pool carries the
4 per-slot bf16 y stores (SWDGE). y is written bf16 and upcast on host
(adds ~0.2% fro error vs the 2e-2 budget). A few warmup matmuls on the
consts blob ramp the PE p-state during the DMA fill.
"""

import math

import numpy as np

D = 1024
H = 64
N = 4096
B = 4
CH = 64  # query chunk size (rows)
NT = 8  # ntiles of 512 keys
NEG = -1e10

_PROG = None  # cached compiled program
_META = None  # cached mask offsets/ends


# ---------------------------------------------------------------- host layout


def _key_order(s: int) -> np.ndarray:
    order = []
    for n in range(NT):
        mine = [8 * n + t for t in range(8) if t % 2 == s]
        theirs = [8 * n + t for t in range(8) if t % 2 != s]
        for c in mine + theirs:
            order.extend(range(CH * c, CH * c + CH))
    return np.array(order)


def _masks(s: int) -> np.ndarray:
    ko = _key_order(s)
    qo = np.array([CH * c + i for c in range(s, 64, 2) for i in range(CH)])
    m = np.zeros((8, 128, 512), dtype=np.float32)
    for r in range(8):
        keys = ko[128 * r : 128 * (r + 1)]
        qs = qo[0:512]
        m[r] = np.where(keys[:, None] <= qs[None, :], 0.0, NEG)
    return m


def _mask_meta():
    """Per masked-tile r (min/max over both parities so the shared program is
    valid for either): OFF_C = 128-floored fully-dead q-prefix, END = end of
    the nonzero mask band."""
    global _META
    if _META is not None:
        return _META
    offs = []
    ends = []
    ms = [_masks(0), _masks(1)]
    for r in range(8):
        offr, endr = [], []
        for s in (0, 1):
            dead = ms[s][r] != 0.0
            colall = dead.all(axis=0)
            off = 0
            while off < 512 and colall[off]:
                off += 1
            anyd = dead.any(axis=1).any()
            cols = np.nonzero(dead.any(axis=0))[0]
            end = int(cols.max()) + 1 if cols.size else 0
            offr.append(off)
            endr.append(end)
        offs.append((min(offr) // 128) * 128)
        ends.append(max(endr))
    _META = (offs, ends)
    return _META


# ---------------------------------------------------------------- bass program

# blobw bf16-column layout: [idb 128 | bias(f32) 4]  (tiny, lands first so
# the PE warmup can start during the x0 fill)
_BW_IDB = 0
_BW_BIAS = 128
_BW_COLS = 132
# blob1 bf16-column layout: [wkv 8x128 | wq 8x64]
_B1_WKV = 0
_B1_WQ = 1024
_B1_COLS = 1536


def _build():
    import concourse.mybir as mybir
    import concourse.tile as tile
    from concourse import bacc

    f32 = mybir.dt.float32
    bf16 = mybir.dt.bfloat16

    OFF_C, ENDS = _mask_meta()

    nc = bacc.Bacc("TRN2", target_bir_lowering=False, debug=False, num_devices=8)

    xh = nc.dram_tensor("xh", [NT, 128, 8, 512], bf16, kind="ExternalInput").ap()
    blobw = nc.dram_tensor("blobw", [128, _BW_COLS], bf16, kind="ExternalInput").ap()
    blob1 = nc.dram_tensor("blob1", [128, _B1_COLS], bf16, kind="ExternalInput").ap()
    maskd = nc.dram_tensor("maskd", [128, 8 * 512], bf16, kind="ExternalInput").ap()
    wobo = nc.dram_tensor("wobo", [65, 1024], bf16, kind="ExternalInput").ap()
    y = nc.dram_tensor("y", [2048, 1024], bf16, kind="ExternalOutput").ap()

    Exp = mybir.ActivationFunctionType.Exp
    mult = mybir.AluOpType.mult
    add_op = mybir.AluOpType.add
    scale = 1.0 / math.sqrt(D)

    with tile.TileContext(nc) as tc:
        with (
            tc.tile_pool(name="consts", bufs=1) as consts,
            tc.tile_pool(name="xp", bufs=3) as xpool,
            tc.tile_pool(name="ep", bufs=6) as epool,
            tc.tile_pool(name="osp", bufs=4) as ospool,
            tc.tile_pool(name="yb", bufs=2) as ypool,
            tc.tile_pool(name="pkv", bufs=1, space="PSUM") as pkv,
            tc.tile_pool(name="pq", bufs=1, space="PSUM") as pq,
            tc.tile_pool(name="po", bufs=2, space="PSUM") as po,
            tc.tile_pool(name="ps", bufs=2, space="PSUM") as ps,
        ):
            # ---- constants. Tiny blobw leads the SP stream (PE warmup
            # dependency); blob1 rides the ACT stream concurrently with x0.
            blobw_sb = consts.tile([128, _BW_COLS], bf16)
            nc.sync.dma_start(blobw_sb[:], blobw[:])
            blob1_sb = consts.tile([128, _B1_COLS], bf16)
            nc.scalar.dma_start(blob1_sb[:], blob1[:])
            wkv_v = blob1_sb[:, _B1_WKV : _B1_WKV + 1024].rearrange(
                "p (c f) -> p c f", c=8
            )
            wq_v = blob1_sb[:, _B1_WQ : _B1_WQ + 512].rearrange("p (c f) -> p c f", c=8)
            idb_v = blobw_sb[:, _BW_IDB : _BW_IDB + 128]
            bias_v = blobw_sb[:, _BW_BIAS : _BW_BIAS + 4].bitcast(f32)

            kvt_sb = consts.tile([128, N], bf16)  # rows 0:64 KT, 64:128 VT
            qt_sb = consts.tile([H, 2048], bf16)
            v65_sb = consts.tile([128, 32 * 65], bf16)
            nc.vector.memset(
                v65_sb[:].rearrange("p (t c) -> p t c", c=65)[:, :, 64:65], 1.0
            )
            mask_sb = consts.tile([128, 8 * 512], bf16)
            wobo_sb = consts.tile([65, 1024], bf16)
            ot_sb = consts.tile([65, 2048], bf16)
            recip_sb = consts.tile([128, 16], f32)
            scratch_sb = consts.tile([1, 8], f32)

            # prewarm the ACT exp table while DMAs stream (ACT stream order:
            # blob1 -> prewarm -> x1 -> masks -> wobo -> x3)
            nc.scalar.activation(
                scratch_sb[:], idb_v[0:1, 0:8], Exp, bias=0.0, scale=1.0
            )

            # masks + wobo ride the Pool DMA stream, which is otherwise idle
            # until the first y store -- they land well before slot 0 needs
            # them and keep the ACT stream short for x1/x3.
            nc.gpsimd.dma_start(
                mask_sb[:].rearrange("p (m f) -> p m f", m=8),
                maskd.rearrange("p (m f) -> p m f", m=8),
            )
            nc.gpsimd.dma_start(wobo_sb[:], wobo[:])

            # PE p-state warmup on blobw data during the x0 fill
            warm = ps.tile([128, 1024], f32, tag="s")
            for _ in range(6):
                nc.tensor.matmul(
                    warm[:, 0:128],
                    idb_v[:],
                    idb_v[:],
                    start=True,
                    stop=True,
                )

            # ---------------- stage A: projections for ntile n
            def emit_a(n):
                xn = xpool.tile([128, 4096], bf16, tag="x")
                xnv = xn[:].rearrange("p (c f) -> p c f", c=8)
                if n == 0:
                    # split first load into quarters so PE starts sooner
                    for qq in range(4):
                        nc.sync.dma_start(
                            xnv[:, 2 * qq : 2 * qq + 2], xh[0][:, 2 * qq : 2 * qq + 2]
                        )
                elif n in (1, 3):
                    nc.scalar.dma_start(xnv[:], xh[n])
                else:
                    nc.sync.dma_start(xnv[:], xh[n])
                # KV first, then the kvt evacuation (DVE) hides under the Q
                # matmuls, and the qt evacuation under the V transposes.
                kvp = pkv.tile([128, 512], f32, tag="kv")
                qp = pq.tile([64, 256], f32, tag="q")
                for c in range(8):
                    nc.tensor.matmul(
                        kvp[:],
                        wkv_v[:, c],
                        xn[:, 512 * c : 512 * c + 512],
                        start=(c == 0),
                        stop=(c == 7),
                    )
                nc.vector.tensor_scalar(
                    out=kvt_sb[:, 512 * n : 512 * (n + 1)],
                    in0=kvp[:],
                    scalar1=bias_v[:, 0:1],
                    scalar2=None,
                    op0=add_op,
                )
                for c in range(8):
                    nc.tensor.matmul(
                        qp[:],
                        wq_v[:, c],
                        xn[:, 512 * c : 512 * c + 256],
                        start=(c == 0),
                        stop=(c == 7),
                    )
                nc.vector.tensor_scalar(
                    out=qt_sb[:, 256 * n : 256 * (n + 1)],
                    in0=qp[:],
                    scalar1=bias_v[0:64, 1:2],
                    scalar2=None,
                    op0=add_op,
                )
                vp = pkv.tile([128, 256], bf16, tag="kv")
                for i, t in enumerate(range(4 * n, 4 * n + 4)):
                    nc.tensor.transpose(
                        vp[:, 64 * i : 64 * (i + 1)],
                        kvt_sb[64:128, 128 * t : 128 * (t + 1)],
                        idb_v[64:128, 64:128],
                    )
                nc.vector.tensor_copy(
                    v65_sb[:].rearrange("p (t c) -> p t c", c=65)[
                        :, 4 * n : 4 * n + 4, 0:64
                    ],
                    vp[:].rearrange("p (t c) -> p t c", c=64),
                )

            # ---------------- per-chunk finish in two phases so the Pool
            # ot copy of chunk c overlaps PE work of the previous phase-2:
            #   phase 1: OT transpose (PE) + ot_sb copy (Pool)
            #   phase 2: D matmuls (PE) + y copy (Pool) + per-slot store
            phase1 = []
            phase2 = []

            def emit_phase1(task):
                j, c, osb, ys = task
                i = 4 * j + c
                pot = pq.tile([65, 128], bf16, tag="q")
                nc.tensor.transpose(pot[:], osb[:], idb_v[:])
                nc.gpsimd.tensor_copy(ot_sb[:, 128 * i : 128 * (i + 1)], pot[:])
                phase2.append(task)

            def emit_phase2(task, split_store=False):
                j, c, _, ys = task
                i = 4 * j + c
                yp = ps.tile([128, 1024], f32, tag="s")
                for d in range(2):
                    nc.tensor.matmul(
                        yp[:, 512 * d : 512 * (d + 1)],
                        ot_sb[:, 128 * i : 128 * (i + 1)],
                        wobo_sb[:, 512 * d : 512 * (d + 1)],
                        start=True,
                        stop=True,
                    )
                nc.gpsimd.tensor_copy(ys[:, 1024 * c : 1024 * (c + 1)], yp[:])
                if split_store:
                    nc.gpsimd.dma_start(
                        y[512 * j + 128 * c : 512 * j + 128 * (c + 1), :],
                        ys[:, 1024 * c : 1024 * (c + 1)],
                    )
                elif c == 3:
                    nc.gpsimd.dma_start(
                        y[512 * j : 512 * (j + 1), :].rearrange(
                            "(t p) d -> p t d", p=128
                        ),
                        ys[:].rearrange("p (t d) -> p t d", t=4),
                    )

            def pop_pending():
                # phase 1 of the next chunk before phase 2 of the previous
                if phase1:
                    emit_phase1(phase1.pop(0))
                elif phase2:
                    emit_phase2(phase2.pop(0))

            # ---------------- stage B + C for slot j
            def emit_slot(j):
                nk = 8 * (j + 1)
                o_ps = po.tile([128, 4 * 65], f32, tag="o")
                ys = ypool.tile([128, 4096], bf16, tag="ys")

                def live(t, c):
                    rr = t - 8 * j
                    return rr < 0 or OFF_C[rr] < 128 * (c + 1)

                # open tiles pair consecutively; masked tiles are paired so
                # both halves share the same causal offset, letting a single
                # strided exp cover the pair
                pairs = [(t, t + 1) for t in range(0, 8 * j, 2)] + [
                    (8 * j + a, 8 * j + b)
                    for a, b in ((0, 2), (1, 3), (4, 6), (5, 7))
                ]
                # PSUM accumulation groups are bank-granular (2KB): all four
                # 65-col chunk accumulators share one bank, so start/stop go
                # on the first/last live matmul in emission order (start's
                # pending-zero covers the full bank).
                lives = [
                    (t, c) for pr in pairs for t in pr for c in range(4) if live(t, c)
                ]
                first_tc, last_tc = lives[0], lives[-1]

                def emit_ct(pr, et, offs):
                    for h in range(2):
                        t = pr[h]
                        for c in range(4):
                            if not live(t, c):
                                continue  # chunk fully causally dead
                            nc.tensor.matmul(
                                o_ps[:, 65 * c : 65 * c + 65],
                                et[:, 512 * h + 128 * c : 512 * h + 128 * (c + 1)],
                                v65_sb[:, 65 * t : 65 * (t + 1)],
                                start=((t, c) == first_tc),
                                stop=((t, c) == last_tc),
                            )

                prevs = []
                for pi, pr in enumerate(pairs):
                    # slot 3 is ACT-throughput-gated on its long open-tile
                    # stretch: spread the D-work pops across all 16 pairs
                    if j < 3 or pi % 2 == 0:
                        pop_pending()
                    sp = ps.tile([128, 1024], f32, tag="s")
                    offs = []
                    for h in range(2):
                        t = pr[h]
                        rr = t - 8 * j
                        off = 0 if rr < 0 else OFF_C[rr]
                        offs.append(off)
                        nc.tensor.matmul(
                            sp[:, 512 * h + off : 512 * (h + 1)],
                            kvt_sb[0:64, 128 * t : 128 * (t + 1)],
                            qt_sb[:, 512 * j + off : 512 * (j + 1)],
                            start=True,
                            stop=(rr < 0),
                        )
                        if rr >= 0:
                            end = ENDS[rr]
                            nc.tensor.matmul(
                                sp[:, 512 * h + off : 512 * h + end],
                                idb_v[:],
                                mask_sb[:, 512 * rr + off : 512 * rr + end],
                                start=False,
                                stop=True,
                            )
                    et = epool.tile([128, 1024], bf16, tag="e")
                    if offs == [0, 0]:
                        nc.scalar.activation(et[:], sp[:], Exp, bias=0.0, scale=scale)
                    elif offs[0] == offs[1]:
                        o = offs[0]
                        nc.scalar.activation(
                            et[:].rearrange("p (b f) -> p b f", b=2)[:, :, o:512],
                            sp[:].rearrange("p (b f) -> p b f", b=2)[:, :, o:512],
                            Exp,
                            bias=0.0,
                            scale=scale,
                        )
                    else:
                        for h in range(2):
                            o = 512 * h + offs[h]
                            nc.scalar.activation(
                                et[:, o : 512 * (h + 1)],
                                sp[:, o : 512 * (h + 1)],
                                Exp,
                                bias=0.0,
                                scale=scale,
                            )
                    prevs.append((pr, et, offs))
                    if len(prevs) > 2:
                        emit_ct(*prevs.pop(0))
                for p in prevs:
                    emit_ct(*p)
                # slot finish: denominators + normalize/evacuate O to bf16
                for c in range(4):
                    i = 4 * j + c
                    nc.vector.reciprocal(
                        recip_sb[:, i : i + 1], o_ps[:, 65 * c + 64 : 65 * c + 65]
                    )
                    osb = ospool.tile([128, 65], bf16, tag="osb")
                    nc.vector.tensor_scalar(
                        out=osb[:],
                        in0=o_ps[:, 65 * c : 65 * c + 65],
                        scalar1=recip_sb[:, i : i + 1],
                        scalar2=None,
                        op0=mult,
                    )
                    phase1.append((j, c, osb, ys))

            for n in range(NT):
                emit_a(n)
                if n % 2 == 1:
                    emit_slot((n - 1) // 2)

            # tail: drain slot 3 with per-chunk stores so the final store is
            # a quarter-slot, shortening the serial tail
            for task in phase1:
                emit_phase1(task)
            phase1 = []
            for task in phase2:
                emit_phase2(task, split_store=True)

    nc.compile()
    return nc


def _get_prog():
    global _PROG
    if _PROG is None:
        _PROG = _build()
    return _PROG


# ---------------------------------------------------------------- host inputs


def _xh(xb, korder):
    """[ntile, partition, chunk, 512] bf16 layout of x[b][korder].T."""
    import ml_dtypes

    xt = xb[korder].T  # [1024, 4096]
    return np.ascontiguousarray(
        xt.reshape(8, 128, 8, 512).transpose(2, 1, 0, 3).astype(ml_dtypes.bfloat16)
    )


def _blobw(bq, bk, bov):
    import ml_dtypes

    blob = np.zeros((128, _BW_COLS), dtype=ml_dtypes.bfloat16)
    blob[:, _BW_IDB : _BW_IDB + 128] = np.eye(128, dtype=ml_dtypes.bfloat16)
    biases = np.zeros((128, 2), dtype=np.float32)
    biases[:, 0] = np.concatenate([bk, bov])
    biases[0:64, 1] = bq
    blob[:, _BW_BIAS : _BW_BIAS + 4] = biases.view(np.uint16).view(ml_dtypes.bfloat16)
    return blob


def _blob1(Wq, Wk, Wov):
    import ml_dtypes

    blob = np.zeros((128, _B1_COLS), dtype=ml_dtypes.bfloat16)
    wkv_t = np.concatenate([Wk, Wov], axis=0).T.astype(ml_dtypes.bfloat16)  # [1024,128]
    blob[:, _B1_WKV : _B1_WKV + 1024] = (
        wkv_t.reshape(8, 128, 128).transpose(1, 0, 2).reshape(128, 1024)
    )
    wq_t = Wq.T.astype(ml_dtypes.bfloat16)  # [1024, 64]
    blob[:, _B1_WQ : _B1_WQ + 512] = (
        wq_t.reshape(8, 128, 64).transpose(1, 0, 2).reshape(128, 512)
    )
    return blob


def _in_map(x, Wq, bq, Wk, bk, Wov, bov, Wo, bo, core):
    import ml_dtypes

    b, s = divmod(core, 2)
    maskd = (
        _masks(s)
        .astype(ml_dtypes.bfloat16)
        .transpose(1, 0, 2)
        .reshape(128, 8 * 512)
    )
    return {
        "xh": _xh(x[b], _key_order(s)),
        "blobw": _blobw(bq, bk, bov),
        "blob1": _blob1(Wq, Wk, Wov),
        "maskd": np.ascontiguousarray(maskd),
        "wobo": np.concatenate([Wo.T, bo[None, :]], axis=0).astype(ml_dtypes.bfloat16),
    }


# ---------------------------------------------------------------- entry point


def kernel(x, Wq, bq, Wk, bk, Wov, bov, Wo, bo, _trace=False):
    from concourse import bass_utils

    x = np.ascontiguousarray(np.asarray(x, dtype=np.float32))
    args = [np.asarray(a, dtype=np.float32) for a in (Wq, bq, Wk, bk, Wov, bov, Wo, bo)]

    nc = _get_prog()
    in_maps = [_in_map(x, *args, core) for core in range(8)]

    res = bass_utils.run_bass_kernel_spmd(
        nc, in_maps, core_ids=list(range(8)), trace=_trace
    )

    y = np.empty((B, N, D), dtype=np.float32)
    for core in range(8):
        b, s = divmod(core, 2)
        yc = np.asarray(res.results[core]["y"]).astype(np.float32)
        y[b].reshape(64, CH, D)[s::2] = yc.reshape(32, CH, D)
    return y


# revision 21
# speedup vs baseline: 1.1180x; 1.0125x over previous
"""Trainium2 Bass kernel for nn_AttentionHead (B=4, n_ctx=4096, d_model=1024,
d_hidden=64, causal, scale=1/sqrt(d_model)).

Sharding: 8 cores = 4 batches x 2 balanced causal shards. Core (b, s) handles
the 2048 query rows in 64-row chunks with chunk%2 == s. Keys/x-columns are
permuted per core (my-parity chunks first within each 512-key ntile) so that
every core runs the IDENTICAL SPMD program:

  - slot j (0..3) = 512 queries = my chunks of ntiles 2j, 2j+1
  - slot j attends k-tiles t = 0..8(j+1)-1 (128 permuted keys each)
  - k-tiles t < 8j are fully open; t = 8j + r (r in 0..7) get an additive
    causal mask that depends only on (r, s) -> 8 mask tiles per core, sent
    as data.

v2 design notes (cost model: matmul cost = moving-dim rows only; DMA
transfers from different issuing engines overlap; same-engine serialize):

  A: KT/VT = [Wk;Wov] @ xT fused (PSUM-accum over 8 d_model chunks), Q
     likewise; PSUM->SBUF copies add biases and downcast to bf16 (bf16
     matmuls are full rate at any N, unlike f32r's N>=256). V transposed
     to natural [k,64] bf16 layout via PE transpose; v65 keeps an
     appended ones column (DVE memset) so E^T@[V|1] also yields the
     softmax denominator.
  B: S^T[k,q] = KT_tile^T @ QT_slot -> PSUM pair tile; additive mask via
     identity matmul for diagonal tiles; exp((S+M)/32) on ACT -> bf16 E.
  C (transposed vs v1): O[q,65] += E_chunk^T @ V65_tile, i.e. E is the
     stationary operand and the 65-wide V65 is moving: 65 rows/chunk-tile
     instead of 512/tile -- less than half the PE cost of v1's C. Col 64
     accumulates the denominator per q-partition.
  D: per 128-q chunk: recip = 1/O[:,64] (DVE), normalize-copy
     O*recip -> bf16 (fused into the mandatory PSUM evacuation; makes
     col 64 exactly 1.0), PE-transpose to OT[65,128], Pool-copy to SBUF,
     then y = OT^T @ [Wo^T; bo] -- the 1.0 row adds bo exactly, so no
     per-element recip multiply is needed after the matmul. y is copied
     to bf16 (DVE/Pool alternating) and stored per-slot.

DMA: three independent streams. SP carries x ntiles (x0 split in half so
PE starts ~1.5us earlier); ACT carries the consts blob + x1/x3 + masks +
wobo (interleaved so each lands just before first use); Pool carries the
4 per-slot bf16 y stores (SWDGE). y is written bf16 and upcast on host
(adds ~0.2% fro error vs the 2e-2 budget). A few warmup matmuls on the
consts blob ramp the PE p-state during the DMA fill.
"""

import math

import numpy as np

D = 1024
H = 64
N = 4096
B = 4
CH = 64  # query chunk size (rows)
NT = 8  # ntiles of 512 keys
NEG = -1e10

_PROG = None  # cached compiled program
_META = None  # cached mask offsets/ends


# ---------------------------------------------------------------- host layout


def _key_order(s: int) -> np.ndarray:
    order = []
    for n in range(NT):
        mine = [8 * n + t for t in range(8) if t % 2 == s]
        theirs = [8 * n + t for t in range(8) if t % 2 != s]
        for c in mine + theirs:
            order.extend(range(CH * c, CH * c + CH))
    return np.array(order)


def _masks(s: int) -> np.ndarray:
    ko = _key_order(s)
    qo = np.array([CH * c + i for c in range(s, 64, 2) for i in range(CH)])
    m = np.zeros((8, 128, 512), dtype=np.float32)
    for r in range(8):
        keys = ko[128 * r : 128 * (r + 1)]
        qs = qo[0:512]
        m[r] = np.where(keys[:, None] <= qs[None, :], 0.0, NEG)
    return m


def _mask_meta():
    """Per masked-tile r (min/max over both parities so the shared program is
    valid for either): OFF_C = 128-floored fully-dead q-prefix, END = end of
    the nonzero mask band."""
    global _META
    if _META is not None:
        return _META
    offs = []
    ends = []
    ms = [_masks(0), _masks(1)]
    for r in range(8):
        offr, endr = [], []
        for s in (0, 1):
            dead = ms[s][r] != 0.0
            colall = dead.all(axis=0)
            off = 0
            while off < 512 and colall[off]:
                off += 1
            anyd = dead.any(axis=1).any()
            cols = np.nonzero(dead.any(axis=0))[0]
            end = int(cols.max()) + 1 if cols.size else 0
            offr.append(off)
            endr.append(end)
        offs.append((min(offr) // 128) * 128)
        ends.append(max(endr))
    _META = (offs, ends)
    return _META


# ---------------------------------------------------------------- bass program

# blobw bf16-column layout: [idb 128 | bias(f32) 4]  (tiny, lands first so
# the PE warmup can start during the x0 fill)
_BW_IDB = 0
_BW_BIAS = 128
_BW_COLS = 132
# blob1 bf16-column layout: [wkv 8x128 | wq 8x64]
_B1_WKV = 0
_B1_WQ = 1024
_B1_COLS = 1536


def _build():
    import concourse.mybir as mybir
    import concourse.tile as tile
    from concourse import bacc

    f32 = mybir.dt.float32
    bf16 = mybir.dt.bfloat16

    OFF_C, ENDS = _mask_meta()

    nc = bacc.Bacc("TRN2", target_bir_lowering=False, debug=False, num_devices=8)

    xh = nc.dram_tensor("xh", [NT, 128, 8, 512], bf16, kind="ExternalInput").ap()
    blobw = nc.dram_tensor("blobw", [128, _BW_COLS], bf16, kind="ExternalInput").ap()
    blob1 = nc.dram_tensor("blob1", [128, _B1_COLS], bf16, kind="ExternalInput").ap()
    maskd = nc.dram_tensor("maskd", [128, 8 * 512], bf16, kind="ExternalInput").ap()
    wobo = nc.dram_tensor("wobo", [65, 1024], bf16, kind="ExternalInput").ap()
    y = nc.dram_tensor("y", [2048, 1024], bf16, kind="ExternalOutput").ap()

    Exp = mybir.ActivationFunctionType.Exp
    mult = mybir.AluOpType.mult
    add_op = mybir.AluOpType.add
    scale = 1.0 / math.sqrt(D)

    with tile.TileContext(nc) as tc:
        with (
            tc.tile_pool(name="consts", bufs=1) as consts,
            tc.tile_pool(name="xp", bufs=3) as xpool,
            tc.tile_pool(name="ep", bufs=6) as epool,
            tc.tile_pool(name="osp", bufs=4) as ospool,
            tc.tile_pool(name="yb", bufs=2) as ypool,
            tc.tile_pool(name="pkv", bufs=1, space="PSUM") as pkv,
            tc.tile_pool(name="pq", bufs=1, space="PSUM") as pq,
            tc.tile_pool(name="po", bufs=2, space="PSUM") as po,
            tc.tile_pool(name="ps", bufs=2, space="PSUM") as ps,
        ):
            # ---- constants. Tiny blobw leads the SP stream (PE warmup
            # dependency); blob1 rides the ACT stream concurrently with x0.
            blobw_sb = consts.tile([128, _BW_COLS], bf16)
            nc.sync.dma_start(blobw_sb[:], blobw[:])
            blob1_sb = consts.tile([128, _B1_COLS], bf16)
            nc.scalar.dma_start(blob1_sb[:], blob1[:])
            wkv_v = blob1_sb[:, _B1_WKV : _B1_WKV + 1024].rearrange(
                "p (c f) -> p c f", c=8
            )
            wq_v = blob1_sb[:, _B1_WQ : _B1_WQ + 512].rearrange("p (c f) -> p c f", c=8)
            idb_v = blobw_sb[:, _BW_IDB : _BW_IDB + 128]
            bias_v = blobw_sb[:, _BW_BIAS : _BW_BIAS + 4].bitcast(f32)

            kvt_sb = consts.tile([128, N], bf16)  # rows 0:64 KT, 64:128 VT
            qt_sb = consts.tile([H, 2048], bf16)
            v65_sb = consts.tile([128, 32 * 65], bf16)
            nc.vector.memset(
                v65_sb[:].rearrange("p (t c) -> p t c", c=65)[:, :, 64:65], 1.0
            )
            mask_sb = consts.tile([128, 8 * 512], bf16)
            wobo_sb = consts.tile([65, 1024], bf16)
            ot_sb = consts.tile([65, 2048], bf16)
            recip_sb = consts.tile([128, 16], f32)
            scratch_sb = consts.tile([1, 8], f32)

            # prewarm the ACT exp table while DMAs stream (ACT stream order:
            # blob1 -> prewarm -> x1 -> masks -> wobo -> x3)
            nc.scalar.activation(
                scratch_sb[:], idb_v[0:1, 0:8], Exp, bias=0.0, scale=1.0
            )

            # masks + wobo ride the Pool DMA stream, which is otherwise idle
            # until the first y store -- they land well before slot 0 needs
            # them and keep the ACT stream short for x1/x3.
            nc.gpsimd.dma_start(
                mask_sb[:].rearrange("p (m f) -> p m f", m=8),
                maskd.rearrange("p (m f) -> p m f", m=8),
            )
            nc.gpsimd.dma_start(wobo_sb[:], wobo[:])



            # ---------------- stage A: projections for ntile n
            def emit_a(n):
                xn = xpool.tile([128, 4096], bf16, tag="x")
                xnv = xn[:].rearrange("p (c f) -> p c f", c=8)
                if n == 0:
                    # split first load into quarters so PE starts sooner
                    for qq in range(4):
                        nc.sync.dma_start(
                            xnv[:, 2 * qq : 2 * qq + 2], xh[0][:, 2 * qq : 2 * qq + 2]
                        )
                elif n in (1, 3):
                    nc.scalar.dma_start(xnv[:], xh[n])
                else:
                    nc.sync.dma_start(xnv[:], xh[n])
                # KV first, then the kvt evacuation (DVE) hides under the Q
                # matmuls, and the qt evacuation under the V transposes.
                kvp = pkv.tile([128, 512], f32, tag="kv")
                qp = pq.tile([64, 256], f32, tag="q")
                for c in range(8):
                    nc.tensor.matmul(
                        kvp[:],
                        wkv_v[:, c],
                        xn[:, 512 * c : 512 * c + 512],
                        start=(c == 0),
                        stop=(c == 7),
                    )
                nc.vector.tensor_scalar(
                    out=kvt_sb[:, 512 * n : 512 * (n + 1)],
                    in0=kvp[:],
                    scalar1=bias_v[:, 0:1],
                    scalar2=None,
                    op0=add_op,
                )
                for c in range(8):
                    nc.tensor.matmul(
                        qp[:],
                        wq_v[:, c],
                        xn[:, 512 * c : 512 * c + 256],
                        start=(c == 0),
                        stop=(c == 7),
                    )
                nc.vector.tensor_scalar(
                    out=qt_sb[:, 256 * n : 256 * (n + 1)],
                    in0=qp[:],
                    scalar1=bias_v[0:64, 1:2],
                    scalar2=None,
                    op0=add_op,
                )
                vp = pkv.tile([128, 256], bf16, tag="kv")
                for i, t in enumerate(range(4 * n, 4 * n + 4)):
                    nc.tensor.transpose(
                        vp[:, 64 * i : 64 * (i + 1)],
                        kvt_sb[64:128, 128 * t : 128 * (t + 1)],
                        idb_v[64:128, 64:128],
                    )
                nc.vector.tensor_copy(
                    v65_sb[:].rearrange("p (t c) -> p t c", c=65)[
                        :, 4 * n : 4 * n + 4, 0:64
                    ],
                    vp[:].rearrange("p (t c) -> p t c", c=64),
                )

            # ---------------- per-chunk finish in two phases so the Pool
            # ot copy of chunk c overlaps PE work of the previous phase-2:
            #   phase 1: OT transpose (PE) + ot_sb copy (Pool)
            #   phase 2: D matmuls (PE) + y copy (Pool) + per-slot store
            phase1 = []
            phase2 = []

            def emit_phase1(task):
                j, c, osb, ys = task
                i = 4 * j + c
                pot = pq.tile([65, 128], bf16, tag="q")
                nc.tensor.transpose(pot[:], osb[:], idb_v[:])
                nc.vector.tensor_copy(ot_sb[:, 128 * i : 128 * (i + 1)], pot[:])
                phase2.append(task)

            def emit_phase2(task, split_store=False):
                j, c, _, ys = task
                i = 4 * j + c
                yp = ps.tile([128, 1024], f32, tag="s")
                for d in range(2):
                    nc.tensor.matmul(
                        yp[:, 512 * d : 512 * (d + 1)],
                        ot_sb[:, 128 * i : 128 * (i + 1)],
                        wobo_sb[:, 512 * d : 512 * (d + 1)],
                        start=True,
                        stop=True,
                    )
                # y evacuation: Pool normally (cheaper than DVE); in the tail
                # alternate DVE/Pool so consecutive chunks' copies overlap
                yeng = nc.vector if (split_store and c % 2 == 1) else nc.gpsimd
                yeng.tensor_copy(ys[:, 1024 * c : 1024 * (c + 1)], yp[:])
                # stores ride the SP DMA stream, idle after the x loads
                if split_store:
                    nc.sync.dma_start(
                        y[512 * j + 128 * c : 512 * j + 128 * (c + 1), :],
                        ys[:, 1024 * c : 1024 * (c + 1)],
                    )
                elif c == 3:
                    nc.sync.dma_start(
                        y[512 * j : 512 * (j + 1), :].rearrange(
                            "(t p) d -> p t d", p=128
                        ),
                        ys[:].rearrange("p (t d) -> p t d", t=4),
                    )

            def pop_pending():
                # phase 1 of the next chunk before phase 2 of the previous
                if phase1:
                    emit_phase1(phase1.pop(0))
                elif phase2:
                    emit_phase2(phase2.pop(0))

            # ---------------- stage B + C for slot j
            def emit_slot(j):
                nk = 8 * (j + 1)
                o_ps = po.tile([128, 4 * 65], f32, tag="o")
                ys = ypool.tile([128, 4096], bf16, tag="ys")

                def live(t, c):
                    rr = t - 8 * j
                    return rr < 0 or OFF_C[rr] < 128 * (c + 1)

                # open tiles pair consecutively; masked tiles are paired so
                # both halves share the same causal offset, letting a single
                # strided exp cover the pair
                pairs = [(t, t + 1) for t in range(0, 8 * j, 2)] + [
                    (8 * j + a, 8 * j + b)
                    for a, b in ((0, 2), (1, 3), (4, 6), (5, 7))
                ]
                # PSUM accumulation groups are bank-granular (2KB): all four
                # 65-col chunk accumulators share one bank, so start/stop go
                # on the first/last live matmul in emission order (start's
                # pending-zero covers the full bank).
                lives = [
                    (t, c) for pr in pairs for t in pr for c in range(4) if live(t, c)
                ]
                first_tc, last_tc = lives[0], lives[-1]

                def emit_ct(pr, et, offs):
                    for h in range(2):
                        t = pr[h]
                        for c in range(4):
                            if not live(t, c):
                                continue  # chunk fully causally dead
                            nc.tensor.matmul(
                                o_ps[:, 65 * c : 65 * c + 65],
                                et[:, 512 * h + 128 * c : 512 * h + 128 * (c + 1)],
                                v65_sb[:, 65 * t : 65 * (t + 1)],
                                start=((t, c) == first_tc),
                                stop=((t, c) == last_tc),
                            )

                prevs = []
                for pi, pr in enumerate(pairs):
                    # slot 3 is ACT-throughput-gated on its long open-tile
                    # stretch: spread the D-work pops across all 16 pairs
                    if j < 3 or pi % 2 == 0:
                        pop_pending()
                    sp = ps.tile([128, 1024], f32, tag="s")
                    offs = []
                    for h in range(2):
                        t = pr[h]
                        rr = t - 8 * j
                        off = 0 if rr < 0 else OFF_C[rr]
                        offs.append(off)
                        nc.tensor.matmul(
                            sp[:, 512 * h + off : 512 * (h + 1)],
                            kvt_sb[0:64, 128 * t : 128 * (t + 1)],
                            qt_sb[:, 512 * j + off : 512 * (j + 1)],
                            start=True,
                            stop=(rr < 0),
                        )
                        if rr >= 0:
                            end = ENDS[rr]
                            nc.tensor.matmul(
                                sp[:, 512 * h + off : 512 * h + end],
                                idb_v[:],
                                mask_sb[:, 512 * rr + off : 512 * rr + end],
                                start=False,
                                stop=True,
                            )
                    et = epool.tile([128, 1024], bf16, tag="e")
                    if offs == [0, 0]:
                        nc.scalar.activation(et[:], sp[:], Exp, bias=0.0, scale=scale)
                    elif offs[0] == offs[1]:
                        o = offs[0]
                        nc.scalar.activation(
                            et[:].rearrange("p (b f) -> p b f", b=2)[:, :, o:512],
                            sp[:].rearrange("p (b f) -> p b f", b=2)[:, :, o:512],
                            Exp,
                            bias=0.0,
                            scale=scale,
                        )
                    else:
                        for h in range(2):
                            o = 512 * h + offs[h]
                            nc.scalar.activation(
                                et[:, o : 512 * (h + 1)],
                                sp[:, o : 512 * (h + 1)],
                                Exp,
                                bias=0.0,
                                scale=scale,
                            )
                    prevs.append((pr, et, offs))
                    if len(prevs) > 2:
                        emit_ct(*prevs.pop(0))
                for p in prevs:
                    emit_ct(*p)
                # slot finish: denominators + normalize/evacuate O to bf16
                for c in range(4):
                    i = 4 * j + c
                    nc.vector.reciprocal(
                        recip_sb[:, i : i + 1], o_ps[:, 65 * c + 64 : 65 * c + 65]
                    )
                    osb = ospool.tile([128, 65], bf16, tag="osb")
                    nc.vector.tensor_scalar(
                        out=osb[:],
                        in0=o_ps[:, 65 * c : 65 * c + 65],
                        scalar1=recip_sb[:, i : i + 1],
                        scalar2=None,
                        op0=mult,
                    )
                    phase1.append((j, c, osb, ys))

            for n in range(NT):
                emit_a(n)
                if n % 2 == 1:
                    emit_slot((n - 1) // 2)

            # tail: drain slot 3 with per-chunk stores so the final store is
            # a quarter-slot, shortening the serial tail
            for task in phase1:
                emit_phase1(task)
            phase1 = []
            for task in phase2:
                emit_phase2(task, split_store=True)

    nc.compile()
    return nc


def _get_prog():
    global _PROG
    if _PROG is None:
        _PROG = _build()
    return _PROG


# ---------------------------------------------------------------- host inputs


def _xh(xb, korder):
    """[ntile, partition, chunk, 512] bf16 layout of x[b][korder].T."""
    import ml_dtypes

    xt = xb[korder].T  # [1024, 4096]
    return np.ascontiguousarray(
        xt.reshape(8, 128, 8, 512).transpose(2, 1, 0, 3).astype(ml_dtypes.bfloat16)
    )


def _blobw(bq, bk, bov):
    import ml_dtypes

    blob = np.zeros((128, _BW_COLS), dtype=ml_dtypes.bfloat16)
    blob[:, _BW_IDB : _BW_IDB + 128] = np.eye(128, dtype=ml_dtypes.bfloat16)
    biases = np.zeros((128, 2), dtype=np.float32)
    biases[:, 0] = np.concatenate([bk, bov])
    biases[0:64, 1] = bq
    blob[:, _BW_BIAS : _BW_BIAS + 4] = biases.view(np.uint16).view(ml_dtypes.bfloat16)
    return blob


def _blob1(Wq, Wk, Wov):
    import ml_dtypes

    blob = np.zeros((128, _B1_COLS), dtype=ml_dtypes.bfloat16)
    wkv_t = np.concatenate([Wk, Wov], axis=0).T.astype(ml_dtypes.bfloat16)  # [1024,128]
    blob[:, _B1_WKV : _B1_WKV + 1024] = (
        wkv_t.reshape(8, 128, 128).transpose(1, 0, 2).reshape(128, 1024)
    )
    wq_t = Wq.T.astype(ml_dtypes.bfloat16)  # [1024, 64]
    blob[:, _B1_WQ : _B1_WQ + 512] = (
        wq_t.reshape(8, 128, 64).transpose(1, 0, 2).reshape(128, 512)
    )
    return blob


def _in_map(x, Wq, bq, Wk, bk, Wov, bov, Wo, bo, core):
    import ml_dtypes

    b, s = divmod(core, 2)
    maskd = (
        _masks(s)
        .astype(ml_dtypes.bfloat16)
        .transpose(1, 0, 2)
        .reshape(128, 8 * 512)
    )
    return {
        "xh": _xh(x[b], _key_order(s)),
        "blobw": _blobw(bq, bk, bov),
        "blob1": _blob1(Wq, Wk, Wov),
        "maskd": np.ascontiguousarray(maskd),
        "wobo": np.concatenate([Wo.T, bo[None, :]], axis=0).astype(ml_dtypes.bfloat16),
    }


# ---------------------------------------------------------------- entry point


def kernel(x, Wq, bq, Wk, bk, Wov, bov, Wo, bo, _trace=False):
    from concourse import bass_utils

    x = np.ascontiguousarray(np.asarray(x, dtype=np.float32))
    args = [np.asarray(a, dtype=np.float32) for a in (Wq, bq, Wk, bk, Wov, bov, Wo, bo)]

    nc = _get_prog()
    in_maps = [_in_map(x, *args, core) for core in range(8)]

    res = bass_utils.run_bass_kernel_spmd(
        nc, in_maps, core_ids=list(range(8)), trace=_trace
    )

    y = np.empty((B, N, D), dtype=np.float32)
    for core in range(8):
        b, s = divmod(core, 2)
        yc = np.asarray(res.results[core]["y"]).astype(np.float32)
        y[b].reshape(64, CH, D)[s::2] = yc.reshape(32, CH, D)
    return y
